# revision 38
# baseline (speedup 1.0000x reference)
"""Trainium2 Bass kernel for nn_BertAttention_78554951843978.

Reference computation (B=2, S=2048, D=1024, H=16, hd=64, fp32):
    q = split_heads(hs @ Wq.T + bq); k = ...; v = ...
    probs = softmax(q k^T / sqrt(64)); ctx = probs @ v
    x = relu(merge_heads(ctx) + hs @ Wp.T)
    out = layernorm(x) * gamma + beta        (eps = 1e-12)

Sharding (8 cores): data-parallel over B (2 groups of 4 cores), tensor-
parallel over heads within a group (4 heads / 256 dims of D per core).

Structure (v3 — overlap-optimized):
  - hsT DMA'd in S-chunks so the first score matmul fires early
  - all matmul operands fp32r (bf16 stationaries cost a separate
    Ldweights instruction on the saturated PE sequencer)
  - attention loop is qn-major (query chunk outer, head-pair inner) so
    layernorm stats for each chunk complete early
  - per-chunk stats AllGather (4 small AGs, pipelined under compute)
    + matmul-based local reduce replaces the terminal AllReduce
  - all partition broadcasts (1/denom, LN scale/shift rows) are K=1/K=2
    ones-matmuls on the PE instead of DRAM DMA bounces
  - per-chunk LN apply + bf16 output DMA, emitted one chunk behind
"""

import numpy as np
import ml_dtypes

import concourse.bass as bass
import concourse.tile as tile
from concourse import mybir
from concourse.bass_utils import run_bass_kernel_spmd

B, S, D, H = 2, 2048, 1024, 16
HD = 64
NCORES = 8
GROUPS = 4          # cores per batch
DC = D // GROUPS    # 256 dims per core
EPS = 1e-12

F32 = mybir.dt.float32
F32R = mybir.dt.float32r
BF16 = mybir.dt.bfloat16
FP8 = mybir.dt.float8e4
VW = 68   # padded head width in vA8 (272B parity stride, 16B-aligned)
AF = mybir.ActivationFunctionType
ALU = mybir.AluOpType

KT = D // 128    # 8 contraction tiles
MT = DC // 128   # 2 output tiles of 128 dims (a head pair each)
NS = S // 512    # 4 query chunks of 512
ST = S // 128    # 16 key tiles of 128

REPLICA_GROUPS = [[0, 1, 2, 3], [4, 5, 6, 7]]


def _split_waits(nc, keep=1):
    """This container's walrus rejects >1 sem wait per (non-EVSEM)
    instruction ("Too many sync wait commands"); hoist extras onto
    preceding single-wait NOPs on the same engine."""
    for bb in nc.main_func.blocks:
        insts = list(bb.instructions)
        out_list = []
        changed = False
        for inst in insts:
            si = inst.sync_info
            cap = 2 if isinstance(inst, mybir.InstEventSemaphore) else keep
            if si is not None and si.on_wait is not None and len(si.on_wait) > cap:
                waits = list(si.on_wait)
                for w in waits[cap:]:
                    out_list.append(mybir.InstNoOp(
                        name=nc.get_next_instruction_name(),
                        engine=inst.engine,
                        ins=[], outs=[],
                        sync_info=mybir.SyncInfo(on_wait=[w], on_update=[]),
                        bass_nofuse=True,
                    ))
                inst.sync_info = mybir.SyncInfo(
                    on_wait=waits[:cap], on_update=list(si.on_update or []))
                changed = True
            out_list.append(inst)
        if changed:
            bb.instructions = out_list


def build_bass():
    nc = bass.Bass(num_devices=NCORES)

    # ---------------- DRAM I/O ----------------
    hsT_d = nc.dram_tensor("hsT", [D, S], BF16, kind="ExternalInput")
    wqT_d = nc.dram_tensor("wqT", [D, DC], BF16, kind="ExternalInput")
    wkT_d = nc.dram_tensor("wkT", [D, DC], BF16, kind="ExternalInput")
    wvT_d = nc.dram_tensor("wvT", [D, DC], BF16, kind="ExternalInput")
    wpT_d = nc.dram_tensor("wpT", [D, DC], BF16, kind="ExternalInput")
    bq_d = nc.dram_tensor("bq", [DC], F32, kind="ExternalInput")
    bk_d = nc.dram_tensor("bk", [DC], F32, kind="ExternalInput")
    bv_d = nc.dram_tensor("bv", [DC], F32, kind="ExternalInput")
    gbr_d = nc.dram_tensor("gbrows", [2, MT, 128], F32R,
                           kind="ExternalInput")
    out_d = nc.dram_tensor("outT", [DC, S], BF16, kind="ExternalOutput")

    lp_cm = nc.allow_low_precision(reason="rel-err budget 2e-2; bf16 ok")
    lp_cm.__enter__()
    with tile.TileContext(nc) as tc:
        with (
            tc.tile_pool(name="persist", bufs=1) as persist,
            tc.tile_pool(name="dram", bufs=1, space="DRAM") as dram,
        ):
            # ------------- persistent SBUF -------------
            qT = persist.tile([128, MT, S], F32R)
            kT = persist.tile([128, MT, S], F32R)
            x = persist.tile([128, MT, S], F32)     # res, x, then relu(x)
            # aug V: [p, s-tile, head, dim|ones]
            vA = persist.tile([128, ST, GROUPS, HD + 1], F32R)
            onesc = persist.tile([128, 1], F32R)             # stats lhsT
            # small constants: cols = bq(2)|bk(2)|eps(1)|beta(2)
            cst = persist.tile([128, 7], F32)
            bq_s, bk_s = cst[:, 0:2], cst[:, 2:4]
            eps_s = cst[:, 4:5]
            bt_s = cst[:, 5:7]
            bv_b = persist.tile([128, DC], F32)              # bv bcast
            gmr = persist.tile([1, MT, 128], F32R)           # gamma row
            oD = persist.tile([4, 1], F32R)                  # 1/D lhsT
            brow = persist.tile([1, 512], F32R)              # -mu*rstd row
            oDF = persist.tile([4, 1], F32)
            onescF = persist.tile([128, 1], F32)

            # DRAM scratch
            scr = dram.tile([MT * NS, 2, 512], F32)     # denom bounce
            cc_in = dram.tile([NS, 2, 512], F32R)
            cc_out = dram.tile([NS, GROUPS, 2, 512], F32R)

            p1sb_cm = tc.tile_pool(name="p1sb", bufs=1)
            p1sb = p1sb_cm.__enter__()
            hsT = p1sb.tile([128, KT, S], F32R)
            wq = p1sb.tile([128, KT, MT, 128], F32R)
            wk = p1sb.tile([128, KT, MT, 128], F32R)
            wv = p1sb.tile([128, KT, DC], F32R)
            wp = p1sb.tile([128, KT, MT, 128], F32R)

            # ---------------- input DMAs ----------------
            # Bulk tensors on the SP queue, ordered so the first score
            # matmul's deps land first. hsT is chunked along S.
            hsT_t = hsT_d.rearrange("(t p) s -> p t s", p=128)
            wq_t = wqT_d.rearrange("(t p) (m f) -> p t m f", p=128, f=128)
            wk_t = wkT_d.rearrange("(t p) (m f) -> p t m f", p=128, f=128)
            wv_t = wvT_d.rearrange("(t p) c -> p t c", p=128)
            wp_t = wpT_d.rearrange("(t p) (m f) -> p t m f", p=128, f=128)

            hsbp_cm = tc.tile_pool(name="hsbp", bufs=2)
            hsbp = hsbp_cm.__enter__()

            def wload(w_sb, src_ap, wb):
                nc.sync.dma_start(out=wb, in_=src_ap)
                nc.vector.tensor_scalar_mul(out=w_sb, in0=wb, scalar1=1.0)

            wb = hsbp.tile([128, KT, DC], BF16, name="hsb")
            wbm = wb.rearrange("p t (m f) -> p t m f", f=128)
            wload(wk[:, :, 0, :], wk_t[:, :, 0, :], wbm[:, :, 0, :])
            wload(wq[:, :, 0, :], wq_t[:, :, 0, :], wbm[:, :, 1, :])
            for sn in range(2):
                ssl = slice(sn * DC, (sn + 1) * DC)
                hsb0 = hsbp.tile([128, KT, DC], BF16, name="hsb")
                nc.sync.dma_start(out=hsb0, in_=hsT_t[:, :, ssl])
                nc.vector.tensor_scalar_mul(
                    out=hsT[:, :, ssl], in0=hsb0, scalar1=1.0)
            wb2 = hsbp.tile([128, KT, DC], BF16, name="hsb")
            wload(wv, wv_t, wb2)

            def load_rest():
                for sn in range(2, 8):
                    sl2 = slice(sn * DC, (sn + 1) * DC)
                    hsb = hsbp.tile([128, KT, DC], BF16, name="hsb")
                    nc.sync.dma_start(out=hsb, in_=hsT_t[:, :, sl2])
                    nc.vector.tensor_scalar_mul(
                        out=hsT[:, :, sl2], in0=hsb, scalar1=1.0)
                wb3 = hsbp.tile([128, KT, DC], BF16, name="hsb")
                wb3m = wb3.rearrange("p t (m f) -> p t m f", f=128)
                wload(wp, wp_t, wb3m)
                wb4 = hsbp.tile([128, KT, DC], BF16, name="hsb")
                wb4m = wb4.rearrange("p t (m f) -> p t m f", f=128)
                wload(wk[:, :, 1, :], wk_t[:, :, 1, :], wb4m[:, :, 0, :])
                wload(wq[:, :, 1, :], wq_t[:, :, 1, :], wb4m[:, :, 1, :])

            # small constants on the gpsimd (SWDGE) queue
            nc.gpsimd.dma_start(
                out=bq_s, in_=bq_d.rearrange("(m p) -> p m", p=128))
            nc.gpsimd.dma_start(
                out=bk_s, in_=bk_d.rearrange("(m p) -> p m", p=128))
            nc.gpsimd.dma_start(out=gmr, in_=gbr_d[0:1, :, :])
            nc.gpsimd.dma_start(
                out=bt_s,
                in_=gbr_d[1:2, :, :].bitcast(F32).rearrange(
                    "r m p -> (r p) m"))
            nc.gpsimd.dma_start(out=bv_b, in_=bass.AP(
                tensor=bv_d[:].tensor, offset=0, ap=[[0, 128], [1, DC]]))
            nc.vector.memset(eps_s, EPS)
            nc.vector.memset(oDF, 1.0 / D)
            nc.vector.memset(onescF, 1.0)
            # f32 -> f32r rounding casts via SWDGE dma (engine memset to an
            # f32r tile fails BIR verification)
            nc.gpsimd.dma_start(out=oD, in_=oDF)
            nc.gpsimd.dma_start(out=onesc, in_=onescF)
            nc.vector.memset(vA[:, :, :, HD:HD + 1].bitcast(F32), 1.0)
            onesr = onesc

            with (
                tc.tile_pool(name="pps", bufs=2, space="PSUM") as pps,
                tc.tile_pool(name="scps", bufs=2, space="PSUM") as scps,
                tc.tile_pool(name="ctxps", bufs=2, space="PSUM") as ctxps,
                tc.tile_pool(name="ptp", bufs=2) as ptp,
                tc.tile_pool(name="small", bufs=1) as small,
                tc.tile_pool(name="stg", bufs=1) as stg,
                tc.tile_pool(name="x2p", bufs=1) as x2p,
            ):
                def proj_group(w_sb, m, n, bias, out_sb):
                    """One [128,512] output block of a W-stationary proj."""
                    ps = pps.tile([128, 512], F32, name="gps")
                    for k in range(KT):
                        nc.tensor.matmul(
                            out=ps, lhsT=w_sb[:, k, m, :],
                            rhs=hsT[:, k, n * 512:(n + 1) * 512],
                            start=(k == 0), stop=(k == KT - 1))
                    o = out_sb[:, m, n * 512:(n + 1) * 512]
                    if bias is not None:
                        nc.vector.tensor_scalar_add(out=o, in0=ps, scalar1=bias)
                    else:
                        # x feeds an fp32r matmul: every write into x must
                        # carry an fp32r output dtype for BIR verification
                        nc.vector.tensor_scalar_add(
                            out=o.bitcast(F32R), in0=ps, scalar1=0.0)

                def v_group(j):
                    """V (natural layout) for s-tile j, hs stationary."""
                    ps = pps.tile([128, 512], F32, name="gps")
                    for k in range(KT):
                        nc.tensor.matmul(
                            out=ps[:, 0:DC],
                            lhsT=hsT[:, k, j * 128:(j + 1) * 128],
                            rhs=wv[:, k, :],
                            start=(k == 0), stop=(k == KT - 1))
                    nc.vector.tensor_add(
                        out=vA[:, j, :, 0:HD],
                        in0=ps[:, 0:DC].rearrange("p (h d) -> p h d", d=HD),
                        in1=bv_b.rearrange("p (h d) -> p h d", d=HD))

                def g_q(m, n):
                    return lambda: proj_group(wq, m, n, bq_s[:, m:m + 1], qT)

                def g_k(m, n):
                    return lambda: proj_group(wk, m, n, bk_s[:, m:m + 1], kT)

                def g_r(m, n):
                    return lambda: proj_group(wp, m, n, None, x)

                def g_v(j):
                    return lambda: v_group(j)

                # upfront: only what the first score matmul needs
                g_q(0, 0)()
                g_k(0, 0)()
                load_rest()

                # filler schedule per (qn, hp) block
                fillers = {
                    (0, 0): [g_v(0), g_v(1), g_v(2), g_v(3), g_k(0, 1),
                             g_v(4), g_v(5), g_v(6), g_v(7), g_k(0, 2),
                             g_v(8), g_v(9), g_v(10), g_v(11), g_k(0, 3),
                             g_v(12), g_v(13), g_v(14), g_v(15), g_r(0, 0),
                             g_q(1, 0), g_k(1, 0)],
                    (0, 1): [g_k(1, 1), g_k(1, 2), g_k(1, 3), g_r(1, 0),
                             g_q(0, 1), g_r(0, 1)],
                    (1, 0): [g_q(1, 1), g_r(1, 1)],
                    (1, 1): [g_q(0, 2), g_r(0, 2)],
                    (2, 0): [g_q(1, 2), g_r(1, 2)],
                    (2, 1): [g_q(0, 3), g_r(0, 3)],
                    (3, 0): [g_q(1, 3), g_r(1, 3)],
                    (3, 1): [],
                }

                def division(hp, qn, ctx0, ctx1):
                    """x[:, hp, qs] += ctx/denom (x holds res).

                    1/denom rows are broadcast down the partitions with a
                    K=1 ones-matmul instead of a DRAM DMA bounce."""
                    qs = slice(qn * 512, (qn + 1) * 512)
                    blk = qn * MT + hp
                    rr = small.tile([1, 1024], F32, name="rr")
                    nc.vector.reciprocal(
                        out=rr[:, 0:512], in_=ctx0[HD:HD + 1, :])
                    nc.vector.reciprocal(
                        out=rr[:, 512:1024], in_=ctx1[HD:HD + 1, :])
                    nc.sync.dma_start(
                        out=bass.AP(tensor=scr.tensor,
                                    offset=scr.offset + blk * 1024,
                                    ap=[[1, 1024]]),
                        in_=rr)
                    rbs = stg.tile([64, 1024], F32, name="rbs")
                    nc.sync.dma_start(
                        out=rbs,
                        in_=bass.AP(tensor=scr.tensor,
                                    offset=scr.offset + blk * 1024,
                                    ap=[[0, 64], [512, 2], [1, 512]]))
                    tmp = stg.tile([128, 512], F32, name="tmp")
                    nc.vector.tensor_mul(
                        out=tmp[0:64, :], in0=ctx0[0:HD, :],
                        in1=rbs[:, 0:512])
                    nc.vector.tensor_mul(
                        out=tmp[64:128, :], in0=ctx1[0:HD, :],
                        in1=rbs[:, 512:1024])
                    nc.vector.tensor_add(
                        out=x[:, hp, qs].bitcast(F32R), in0=x[:, hp, qs],
                        in1=tmp)

                def attention_block(hp, qn):
                    qs = slice(qn * 512, (qn + 1) * 512)
                    ctx0 = ctxps.tile([128, 512], F32, name="ctx")
                    ctx1 = ctxps.tile([128, 512], F32, name="ctx")
                    fl = list(fillers[(qn, hp)])
                    # spread fillers evenly over the 16 ks slots
                    per_slot = [0] * ST
                    for i in range(len(fl)):
                        per_slot[(i * ST) // max(1, len(fl))] += 1
                    fl.reverse()

                    def ctx_mms(pt, ks):
                        nc.tensor.matmul(
                            out=ctx0[0:HD + 1, :],
                            lhsT=vA[:, ks, 2 * hp, :],
                            rhs=pt[:, 0:512],
                            start=(ks == 0), stop=(ks == ST - 1))
                        nc.tensor.matmul(
                            out=ctx1[0:HD + 1, :],
                            lhsT=vA[:, ks, 2 * hp + 1, :],
                            rhs=pt[:, 512:1024],
                            start=(ks == 0), stop=(ks == ST - 1))

                    prev = None
                    for ks in range(ST):
                        sc = scps.tile([128, 1024], F32, name="sc")
                        kslc = slice(ks * 128, (ks + 1) * 128)
                        nc.tensor.matmul(
                            out=sc[:, 0:512],
                            lhsT=kT[0:64, hp, kslc],
                            rhs=qT[0:64, hp, qs])
                        nc.tensor.matmul(
                            out=sc[:, 512:1024],
                            lhsT=kT[64:128, hp, kslc],
                            rhs=qT[64:128, hp, qs])
                        pt = ptp.tile([128, 1024], F32R, name="pt")
                        nc.scalar.activation(
                            out=pt, in_=sc, func=AF.Exp,
                            scale=float(1.0 / np.sqrt(HD)))
                        for _ in range(per_slot[ks]):
                            if fl:
                                fl.pop()()
                        if prev is not None:
                            ctx_mms(*prev)
                        prev = (pt, ks)
                    ctx_mms(*prev)
                    division(hp, qn, ctx0, ctx1)

                stats_ctx = {}

                def stats_part(qn, t):
                    """relu + square + stats-matmul contribution of head
                    pair t, emitted right after division(t, qn). The psum
                    rows are merged into the SBUF accumulator immediately
                    so the psum pool is never held across a block."""
                    qs = slice(qn * 512, (qn + 1) * 512)
                    if t == 0:
                        stats_ctx[qn] = (
                            small.tile([1, 1024], F32R, name="st"),)
                    (st,) = stats_ctx[qn]
                    x2 = x2p.tile([128, 512], F32R, name="x2")
                    nc.vector.tensor_scalar_max(
                        out=x[:, t, qs].bitcast(F32R), in0=x[:, t, qs],
                        scalar1=0.0)
                    nc.scalar.activation(
                        out=x2, in_=x[:, t, qs], func=AF.Square)
                    sp = pps.tile([128, 512], F32, name="gps")
                    sq = pps.tile([128, 512], F32, name="gps")
                    nc.tensor.matmul(
                        out=sp[0:1, :], lhsT=onesr,
                        rhs=x[:, t, qs].bitcast(F32R))
                    nc.tensor.matmul(
                        out=sq[0:1, :], lhsT=onesr, rhs=x2)
                    if t == 0:
                        nc.vector.tensor_scalar_mul(
                            out=st[:, 0:512], in0=sp[0:1, :], scalar1=1.0)
                        nc.vector.tensor_scalar_mul(
                            out=st[:, 512:1024], in0=sq[0:1, :],
                            scalar1=1.0)
                    else:
                        nc.vector.tensor_add(
                            out=st[:, 0:512],
                            in0=st[:, 0:512].bitcast(F32),
                            in1=sp[0:1, :])
                        nc.vector.tensor_add(
                            out=st[:, 512:1024],
                            in0=st[:, 512:1024].bitcast(F32),
                            in1=sq[0:1, :])

                def stats(qn):
                    """bounce accumulated stats to DRAM + AllGather."""
                    (st,) = stats_ctx.pop(qn)
                    nc.sync.dma_start(
                        out=bass.AP(tensor=cc_in.tensor,
                                    offset=cc_in.offset + qn * 1024,
                                    ap=[[1, 1024]]),
                        in_=st)
                    nc.gpsimd.collective_compute(
                        "AllGather", ALU.bypass,
                        replica_groups=REPLICA_GROUPS,
                        ins=[cc_in[qn].opt()], outs=[cc_out[qn].opt()],
                    )

                def apply_ln(qn):
                    """Reduce gathered stats (matmul), row math on [1,512],
                    broadcast gamma*A / gamma*B+beta rows (matmuls), apply,
                    DMA out."""
                    qs = slice(qn * 512, (qn + 1) * 512)
                    cc_sb = stg.tile([4, 1024], F32R, name="ccsb")
                    nc.sync.dma_start(out=cc_sb, in_=cc_out[qn].rearrange(
                        "c v s -> c (v s)"))
                    stt = pps.tile([128, 512], F32, name="gps")
                    stq = pps.tile([128, 512], F32, name="gps")
                    nc.tensor.matmul(
                        out=stt[0:1, :], lhsT=oD,
                        rhs=cc_sb[:, 0:512])
                    nc.tensor.matmul(
                        out=stq[0:1, :], lhsT=oD,
                        rhs=cc_sb[:, 512:1024])
                    # row math on [1, 512]: stt[0]=mu, stt[32]=E[x^2];
                    # rm is reused in place down the chain
                    rm = small.tile([1, 512], F32, name="rm")
                    nc.scalar.activation(
                        out=rm, in_=stt[0:1, :], func=AF.Square)
                    nc.vector.scalar_tensor_tensor(
                        out=rm, in0=stq[0:1, :], scalar=1.0,
                        in1=rm, op0=ALU.mult, op1=ALU.subtract)
                    nc.scalar.activation(
                        out=rm, in_=rm, func=AF.Sqrt, bias=eps_s[0:1, :])
                    arow = small.tile([1, 512], F32R, name="arow")
                    nc.vector.reciprocal(out=arow, in_=rm)
                    # brow = -mu * rstd
                    nc.vector.scalar_tensor_tensor(
                        out=brow, in0=stt[0:1, :], scalar=-1.0,
                        in1=arow.bitcast(F32), op0=ALU.mult, op1=ALU.mult)
                    # broadcast rows with gamma/beta folded in:
                    #   ab[:,0:512] = gamma[p]*A[s]; ab[:,512:]=gamma*B+beta
                    ot = stg.tile([128, MT, 512], BF16, name="ot")
                    for t in range(MT):
                        abA = pps.tile([128, 512], F32, name="gps")
                        abB = pps.tile([128, 512], F32, name="gps")
                        nc.tensor.matmul(
                            out=abA, lhsT=gmr[:, t, :], rhs=arow)
                        nc.tensor.matmul(
                            out=abB, lhsT=gmr[:, t, :], rhs=brow)
                        ota = stg.tile([128, 512], F32, name="tmp")
                        nc.vector.tensor_mul(
                            out=ota, in0=x[:, t, qs], in1=abA)
                        # ot = (ota + beta) + gamma*brow_bcast
                        nc.vector.scalar_tensor_tensor(
                            out=ot[:, t, :], in0=ota,
                            scalar=bt_s[:, t:t + 1], in1=abB,
                            op0=ALU.add, op1=ALU.add)
                    out_t = out_d.rearrange("(t p) s -> p t s", p=128)
                    nc.sync.dma_start(out=out_t[:, :, qs], in_=ot)

                # ================= main loop =================
                for qn in range(NS):
                    for hp in range(MT):
                        attention_block(hp, qn)
                        stats_part(qn, hp)
                    stats(qn)
                    if qn == 1:
                        apply_ln(0)
                # applies 1 and 2 run inside AllGather(3)'s flight window
                apply_ln(1)
                apply_ln(2)
                apply_ln(NS - 1)
            hsbp_cm.__exit__(None, None, None)
            p1sb_cm.__exit__(None, None, None)
    lp_cm.__exit__(None, None, None)
    _split_waits(nc)
    return nc


_NC = None
LAST_RESULT = None


def _get_nc():
    global _NC
    if _NC is None:
        _NC = build_bass()
    return _NC


def kernel(hidden_states, Wq, bq, Wk, bk, Wv, bv, Wp, gamma, beta):
    hs = np.ascontiguousarray(np.asarray(hidden_states, dtype=np.float32))
    Wq = np.asarray(Wq, np.float32)
    Wk = np.asarray(Wk, np.float32)
    Wv = np.asarray(Wv, np.float32)
    Wp = np.asarray(Wp, np.float32)
    bq = np.asarray(bq, np.float32)
    bk = np.asarray(bk, np.float32)
    bv = np.asarray(bv, np.float32)
    gamma = np.asarray(gamma, np.float32)
    beta = np.asarray(beta, np.float32)

    nc = _get_nc()
    in_maps = []
    for c in range(NCORES):
        b, g = divmod(c, GROUPS)
        sl = slice(g * DC, (g + 1) * DC)
        gb = np.stack([gamma[sl].reshape(MT, 128),
                       beta[sl].reshape(MT, 128)])  # [2, MT, 128]
        in_maps.append({
            "hsT": np.ascontiguousarray(hs[b].T).astype(
                ml_dtypes.bfloat16),
            "wqT": np.ascontiguousarray(Wq[sl].T).astype(ml_dtypes.bfloat16),
            "wkT": np.ascontiguousarray(Wk[sl].T).astype(ml_dtypes.bfloat16),
            "wvT": np.ascontiguousarray(Wv[sl].T).astype(ml_dtypes.bfloat16),
            "wpT": np.ascontiguousarray(Wp[sl].T).astype(ml_dtypes.bfloat16),
            "bq": np.ascontiguousarray(bq[sl]),
            "bk": np.ascontiguousarray(bk[sl]),
            "bv": np.ascontiguousarray(bv[sl]),
            "gbrows": np.ascontiguousarray(gb),
        })
    res = run_bass_kernel_spmd(nc, in_maps, core_ids=list(range(NCORES)))
    global LAST_RESULT
    LAST_RESULT = res
    out = np.empty((B, S, D), np.float32)
    for c, r in enumerate(res.results):
        b, g = divmod(c, GROUPS)
        out[b, :, g * DC:(g + 1) * DC] = np.asarray(
            r["outT"]).astype(np.float32).T
    return out


# revision 44
# speedup vs baseline: 1.1797x; 1.1797x over previous
"""Trainium2 Bass kernel for nn_BertAttention_78554951843978.

Reference computation (B=2, S=2048, D=1024, H=16, hd=64, fp32):
    q = split_heads(hs @ Wq.T + bq); k = ...; v = ...
    probs = softmax(q k^T / sqrt(64)); ctx = probs @ v
    x = relu(merge_heads(ctx) + hs @ Wp.T)
    out = layernorm(x) * gamma + beta        (eps = 1e-12)

Sharding (8 cores): data-parallel over B (2 groups of 4 cores), tensor-
parallel over heads within a group (4 heads / 256 dims of D per core).

Structure (v3 — overlap-optimized):
  - hsT DMA'd in S-chunks so the first score matmul fires early
  - all matmul operands fp32r (bf16 stationaries cost a separate
    Ldweights instruction on the saturated PE sequencer)
  - attention loop is qn-major (query chunk outer, head-pair inner) so
    layernorm stats for each chunk complete early
  - per-chunk stats AllGather (4 small AGs, pipelined under compute)
    + matmul-based local reduce replaces the terminal AllReduce
  - all partition broadcasts (1/denom, LN scale/shift rows) are K=1/K=2
    ones-matmuls on the PE instead of DRAM DMA bounces
  - per-chunk LN apply + bf16 output DMA, emitted one chunk behind
"""

import numpy as np
import ml_dtypes

import concourse.bass as bass
import concourse.tile as tile
from concourse import mybir
from concourse.bass_utils import run_bass_kernel_spmd

B, S, D, H = 2, 2048, 1024, 16
HD = 64
NCORES = 8
GROUPS = 4          # cores per batch
DC = D // GROUPS    # 256 dims per core
EPS = 1e-12

F32 = mybir.dt.float32
F32R = mybir.dt.float32r
BF16 = mybir.dt.bfloat16
FP8 = mybir.dt.float8e4
VW = 68   # padded head width in vA8 (272B parity stride, 16B-aligned)
AF = mybir.ActivationFunctionType
ALU = mybir.AluOpType

KT = D // 128    # 8 contraction tiles
MT = DC // 128   # 2 output tiles of 128 dims (a head pair each)
NS = S // 512    # 4 query chunks of 512
ST = S // 128    # 16 key tiles of 128

REPLICA_GROUPS = [[0, 1, 2, 3], [4, 5, 6, 7]]


def _split_waits(nc, keep=1):
    """This container's walrus rejects >1 sem wait per (non-EVSEM)
    instruction ("Too many sync wait commands"); hoist extras onto
    preceding single-wait NOPs on the same engine."""
    for bb in nc.main_func.blocks:
        insts = list(bb.instructions)
        out_list = []
        changed = False
        for inst in insts:
            si = inst.sync_info
            cap = 2 if isinstance(inst, mybir.InstEventSemaphore) else keep
            if si is not None and si.on_wait is not None and len(si.on_wait) > cap:
                waits = list(si.on_wait)
                for w in waits[cap:]:
                    out_list.append(mybir.InstNoOp(
                        name=nc.get_next_instruction_name(),
                        engine=inst.engine,
                        ins=[], outs=[],
                        sync_info=mybir.SyncInfo(on_wait=[w], on_update=[]),
                        bass_nofuse=True,
                    ))
                inst.sync_info = mybir.SyncInfo(
                    on_wait=waits[:cap], on_update=list(si.on_update or []))
                changed = True
            out_list.append(inst)
        if changed:
            bb.instructions = out_list


def build_bass():
    nc = bass.Bass(num_devices=NCORES)

    # ---------------- DRAM I/O ----------------
    hsT_d = nc.dram_tensor("hsT", [D, S], BF16, kind="ExternalInput")
    wqT_d = nc.dram_tensor("wqT", [D, DC], BF16, kind="ExternalInput")
    wkT_d = nc.dram_tensor("wkT", [D, DC], BF16, kind="ExternalInput")
    wvT_d = nc.dram_tensor("wvT", [D, DC], BF16, kind="ExternalInput")
    wpT_d = nc.dram_tensor("wpT", [D, DC], BF16, kind="ExternalInput")
    bq_d = nc.dram_tensor("bq", [DC], F32, kind="ExternalInput")
    bk_d = nc.dram_tensor("bk", [DC], F32, kind="ExternalInput")
    bvh_d = nc.dram_tensor("bv", [DC], BF16, kind="ExternalInput")
    gbr_d = nc.dram_tensor("gbrows", [2, MT, 128], F32R,
                           kind="ExternalInput")
    out_d = nc.dram_tensor("outT", [DC, S], BF16, kind="ExternalOutput")

    lp_cm = nc.allow_low_precision(reason="rel-err budget 2e-2; bf16 ok")
    lp_cm.__enter__()
    with tile.TileContext(nc) as tc:
        with (
            tc.tile_pool(name="persist", bufs=1) as persist,
            tc.tile_pool(name="dram", bufs=1, space="DRAM") as dram,
        ):
            # ------------- persistent SBUF -------------
            qT = persist.tile([128, MT, S], F32R)
            kT = persist.tile([128, MT, S], F32R)
            x = persist.tile([128, MT, S], F32)     # res, x, then relu(x)
            # aug V: [p, s-tile, head, dim|ones]
            vA = persist.tile([128, ST, GROUPS, HD + 1], F32R)
            onesc = persist.tile([128, 1], F32R)             # stats lhsT
            # small constants: cols = bq(2)|bk(2)|eps(1)|beta(2)|1.0|1/D
            cst = persist.tile([128, 9], F32)
            bq_s, bk_s = cst[:, 0:2], cst[:, 2:4]
            eps_s = cst[:, 4:5]
            bt_s = cst[:, 5:7]
            bv_b = persist.tile([128, DC], BF16)             # bv bcast
            gmr = persist.tile([1, MT, 128], F32R)           # gamma row
            oD = persist.tile([4, 1], F32R)                  # 1/D lhsT
            brow = persist.tile([1, 512], F32R)              # -mu*rstd row

            # DRAM scratch
            scr = dram.tile([MT * NS, 2, 512], F32)     # denom bounce
            cc_in = dram.tile([NS, 2, 512], F32R)
            cc_out = dram.tile([NS, GROUPS, 2, 512], F32R)

            p1sb_cm = tc.tile_pool(name="p1sb", bufs=1)
            p1sb = p1sb_cm.__enter__()
            hsT = p1sb.tile([128, KT, S], F32R)
            wq = p1sb.tile([128, KT, MT, 128], F32R)
            wk = p1sb.tile([128, KT, MT, 128], F32R)
            wv = p1sb.tile([128, KT, DC], F32R)
            wp = p1sb.tile([128, KT, MT, 128], F32R)

            # ---------------- input DMAs ----------------
            # Bulk tensors on the SP queue, ordered so the first score
            # matmul's deps land first. hsT is chunked along S.
            hsT_t = hsT_d.rearrange("(t p) s -> p t s", p=128)
            wq_t = wqT_d.rearrange("(t p) (m f) -> p t m f", p=128, f=128)
            wk_t = wkT_d.rearrange("(t p) (m f) -> p t m f", p=128, f=128)
            wv_t = wvT_d.rearrange("(t p) c -> p t c", p=128)
            wp_t = wpT_d.rearrange("(t p) (m f) -> p t m f", p=128, f=128)

            hsbp_cm = tc.tile_pool(name="hsbp", bufs=2)
            hsbp = hsbp_cm.__enter__()

            def wload(w_sb, src_ap, wb):
                nc.sync.dma_start(out=wb, in_=src_ap)
                nc.vector.tensor_scalar_mul(out=w_sb, in0=wb, scalar1=1.0)

            CW = 128

            def hs_chunk(sn):
                sl2 = slice(sn * CW, (sn + 1) * CW)
                hsb = hsbp.tile([128, KT, CW], BF16, name="hsb")
                nc.sync.dma_start(out=hsb, in_=hsT_t[:, :, sl2])
                nc.vector.tensor_scalar_mul(
                    out=hsT[:, :, sl2], in0=hsb, scalar1=1.0)

            wload(wk[:, :, 0, :], wk_t[:, :, 0, :],
                  hsbp.tile([128, KT, CW], BF16, name="hsb"))
            wload(wq[:, :, 0, :], wq_t[:, :, 0, :],
                  hsbp.tile([128, KT, CW], BF16, name="hsb"))
            for sn in range(4):
                hs_chunk(sn)
            wload(wv[:, :, 0:128], wv_t[:, :, 0:128],
                  hsbp.tile([128, KT, CW], BF16, name="hsb"))
            wload(wv[:, :, 128:256], wv_t[:, :, 128:256],
                  hsbp.tile([128, KT, CW], BF16, name="hsb"))

            def load_rest():
                for sn in range(4, 16):
                    hs_chunk(sn)
                wload(wp[:, :, 0, :], wp_t[:, :, 0, :],
                      hsbp.tile([128, KT, CW], BF16, name="hsb"))
                wload(wp[:, :, 1, :], wp_t[:, :, 1, :],
                      hsbp.tile([128, KT, CW], BF16, name="hsb"))
                wload(wk[:, :, 1, :], wk_t[:, :, 1, :],
                      hsbp.tile([128, KT, CW], BF16, name="hsb"))
                wload(wq[:, :, 1, :], wq_t[:, :, 1, :],
                      hsbp.tile([128, KT, CW], BF16, name="hsb"))

            # small constants on the gpsimd (SWDGE) queue
            nc.gpsimd.dma_start(
                out=bq_s, in_=bq_d.rearrange("(m p) -> p m", p=128))
            nc.gpsimd.dma_start(
                out=bk_s, in_=bk_d.rearrange("(m p) -> p m", p=128))
            nc.gpsimd.dma_start(out=gmr, in_=gbr_d[0:1, :, :])
            nc.gpsimd.dma_start(
                out=bt_s,
                in_=gbr_d[1:2, :, :].bitcast(F32).rearrange(
                    "r m p -> (r p) m"))
            nc.gpsimd.dma_start(out=bv_b, in_=bass.AP(
                tensor=bvh_d[:].tensor, offset=0, ap=[[0, 128], [1, DC]]))
            nc.vector.memset(eps_s, EPS)
            nc.vector.memset(cst[:, 7:8], 1.0)
            nc.vector.memset(cst[:, 8:9], 1.0 / D)
            # f32 -> f32r rounding casts via SWDGE dma (engine memset to an
            # f32r tile fails BIR verification)
            nc.gpsimd.dma_start(out=oD, in_=cst[0:4, 8:9])
            nc.gpsimd.dma_start(out=onesc, in_=cst[:, 7:8])
            nc.vector.memset(vA[:, :, :, HD:HD + 1].bitcast(F32), 1.0)
            onesr = onesc

            with (
                tc.tile_pool(name="pps", bufs=2, space="PSUM") as pps,
                tc.tile_pool(name="scps", bufs=2, space="PSUM") as scps,
                tc.tile_pool(name="ctxps", bufs=2, space="PSUM") as ctxps,
                tc.tile_pool(name="ptp", bufs=2) as ptp,
                tc.tile_pool(name="small", bufs=1) as small,
                tc.tile_pool(name="stg", bufs=1) as stg,
                tc.tile_pool(name="x2p", bufs=1) as x2p,
            ):
                def proj_group(w_sb, m, n, bias, out_sb):
                    """One [128,512] output block of a W-stationary proj."""
                    ps = pps.tile([128, 512], F32, name="gps")
                    for k in range(KT):
                        nc.tensor.matmul(
                            out=ps, lhsT=w_sb[:, k, m, :],
                            rhs=hsT[:, k, n * 512:(n + 1) * 512],
                            start=(k == 0), stop=(k == KT - 1))
                    o = out_sb[:, m, n * 512:(n + 1) * 512]
                    if bias is not None:
                        nc.vector.tensor_scalar_add(out=o, in0=ps, scalar1=bias)
                    else:
                        # x feeds an fp32r matmul: every write into x must
                        # carry an fp32r output dtype for BIR verification
                        nc.vector.tensor_scalar_add(
                            out=o.bitcast(F32R), in0=ps, scalar1=0.0)

                def v_group(j):
                    """V (natural layout) for s-tile j, hs stationary."""
                    ps = pps.tile([128, 512], F32, name="gps")
                    for k in range(KT):
                        nc.tensor.matmul(
                            out=ps[:, 0:DC],
                            lhsT=hsT[:, k, j * 128:(j + 1) * 128],
                            rhs=wv[:, k, :],
                            start=(k == 0), stop=(k == KT - 1))
                    nc.vector.tensor_add(
                        out=vA[:, j, :, 0:HD],
                        in0=ps[:, 0:DC].rearrange("p (h d) -> p h d", d=HD),
                        in1=bv_b.rearrange("p (h d) -> p h d", d=HD))

                def g_q(m, n):
                    return lambda: proj_group(wq, m, n, bq_s[:, m:m + 1], qT)

                def g_k(m, n):
                    return lambda: proj_group(wk, m, n, bk_s[:, m:m + 1], kT)

                def g_r(m, n):
                    return lambda: proj_group(wp, m, n, None, x)

                def g_v(j):
                    return lambda: v_group(j)

                # upfront: only what the first score matmul needs
                g_q(0, 0)()
                g_k(0, 0)()
                load_rest()

                # filler schedule per (qn, hp) block
                fillers = {
                    (0, 0): [g_v(0), g_v(1), g_v(2), g_v(3), g_k(0, 1),
                             g_v(4), g_v(5), g_v(6), g_v(7), g_k(0, 2),
                             g_v(8), g_v(9), g_v(10), g_v(11), g_k(0, 3),
                             g_v(12), g_v(13), g_v(14), g_v(15), g_r(0, 0),
                             g_q(1, 0), g_k(1, 0)],
                    (0, 1): [g_k(1, 1), g_k(1, 2), g_k(1, 3), g_r(1, 0),
                             g_q(0, 1), g_r(0, 1)],
                    (1, 0): [g_q(1, 1), g_r(1, 1)],
                    (1, 1): [g_q(0, 2), g_r(0, 2)],
                    (2, 0): [g_q(1, 2), g_r(1, 2)],
                    (2, 1): [g_q(0, 3), g_r(0, 3)],
                    (3, 0): [g_q(1, 3), g_r(1, 3)],
                    (3, 1): [],
                }

                def division(hp, qn, ctx0, ctx1):
                    """x[:, hp, qs] += ctx/denom (x holds res).

                    1/denom rows are broadcast down the partitions with a
                    K=1 ones-matmul instead of a DRAM DMA bounce."""
                    qs = slice(qn * 512, (qn + 1) * 512)
                    blk = qn * MT + hp
                    ctxc = stg.tile([128, 1024], F32, name="ctxc")
                    nc.vector.tensor_copy(
                        out=ctxc[0:HD + 1, 0:512], in_=ctx0[0:HD + 1, :])
                    nc.vector.tensor_copy(
                        out=ctxc[0:HD + 1, 512:1024], in_=ctx1[0:HD + 1, :])
                    rr = small.tile([1, 1024], F32, name="rr")
                    nc.vector.reciprocal(
                        out=rr[:, 0:512], in_=ctxc[HD:HD + 1, 0:512])
                    nc.vector.reciprocal(
                        out=rr[:, 512:1024], in_=ctxc[HD:HD + 1, 512:1024])
                    nc.sync.dma_start(
                        out=bass.AP(tensor=scr.tensor,
                                    offset=scr.offset + blk * 1024,
                                    ap=[[1, 1024]]),
                        in_=rr)
                    rbs = stg.tile([64, 1024], F32, name="rbs")
                    nc.sync.dma_start(
                        out=rbs,
                        in_=bass.AP(tensor=scr.tensor,
                                    offset=scr.offset + blk * 1024,
                                    ap=[[0, 64], [512, 2], [1, 512]]))
                    tmp = stg.tile([128, 512], F32, name="tmp")
                    nc.vector.tensor_mul(
                        out=tmp[0:64, :], in0=ctxc[0:HD, 0:512],
                        in1=rbs[:, 0:512])
                    nc.vector.tensor_mul(
                        out=tmp[64:128, :], in0=ctxc[0:HD, 512:1024],
                        in1=rbs[:, 512:1024])
                    nc.vector.tensor_add(
                        out=x[:, hp, qs].bitcast(F32R), in0=x[:, hp, qs],
                        in1=tmp)

                def attention_block(hp, qn):
                    qs = slice(qn * 512, (qn + 1) * 512)
                    ctx0 = ctxps.tile([128, 512], F32, name="ctx")
                    ctx1 = ctxps.tile([128, 512], F32, name="ctx")
                    fl = list(fillers[(qn, hp)])
                    # spread fillers evenly over the 16 ks slots
                    per_slot = [0] * ST
                    for i in range(len(fl)):
                        per_slot[(i * ST) // max(1, len(fl))] += 1
                    fl.reverse()

                    def ctx_mms(pt, ks):
                        nc.tensor.matmul(
                            out=ctx0[0:HD + 1, :],
                            lhsT=vA[:, ks, 2 * hp, :],
                            rhs=pt[:, 0:512],
                            start=(ks == 0), stop=(ks == ST - 1))
                        nc.tensor.matmul(
                            out=ctx1[0:HD + 1, :],
                            lhsT=vA[:, ks, 2 * hp + 1, :],
                            rhs=pt[:, 512:1024],
                            start=(ks == 0), stop=(ks == ST - 1))

                    prev = None
                    for ks in range(ST):
                        sc = scps.tile([128, 1024], F32, name="sc")
                        kslc = slice(ks * 128, (ks + 1) * 128)
                        nc.tensor.matmul(
                            out=sc[:, 0:512],
                            lhsT=kT[0:64, hp, kslc],
                            rhs=qT[0:64, hp, qs])
                        nc.tensor.matmul(
                            out=sc[:, 512:1024],
                            lhsT=kT[64:128, hp, kslc],
                            rhs=qT[64:128, hp, qs])
                        pt = ptp.tile([128, 1024], F32R, name="pt")
                        nc.scalar.activation(
                            out=pt, in_=sc, func=AF.Exp,
                            scale=float(1.0 / np.sqrt(HD)))
                        for _ in range(per_slot[ks]):
                            if fl:
                                fl.pop()()
                        if prev is not None:
                            ctx_mms(*prev)
                        prev = (pt, ks)
                    ctx_mms(*prev)
                    division(hp, qn, ctx0, ctx1)

                stats_ctx = {}

                def stats_part(qn, t):
                    """relu + square + stats-matmul contribution of head
                    pair t, emitted right after division(t, qn). The psum
                    rows are merged into the SBUF accumulator immediately
                    so the psum pool is never held across a block."""
                    qs = slice(qn * 512, (qn + 1) * 512)
                    if t == 0:
                        stats_ctx[qn] = (
                            small.tile([1, 1024], F32R, name="st"),)
                    (st,) = stats_ctx[qn]
                    x2 = x2p.tile([128, 512], F32R, name="x2")
                    nc.vector.tensor_scalar_max(
                        out=x[:, t, qs].bitcast(F32R), in0=x[:, t, qs],
                        scalar1=0.0)
                    nc.scalar.activation(
                        out=x2, in_=x[:, t, qs], func=AF.Square)
                    sp = pps.tile([128, 512], F32, name="gps")
                    sq = pps.tile([128, 512], F32, name="gps")
                    nc.tensor.matmul(
                        out=sp[0:1, :], lhsT=onesr,
                        rhs=x[:, t, qs].bitcast(F32R))
                    nc.tensor.matmul(
                        out=sq[0:1, :], lhsT=onesr, rhs=x2)
                    if t == 0:
                        nc.vector.tensor_scalar_mul(
                            out=st[:, 0:512], in0=sp[0:1, :], scalar1=1.0)
                        nc.vector.tensor_scalar_mul(
                            out=st[:, 512:1024], in0=sq[0:1, :],
                            scalar1=1.0)
                    else:
                        nc.vector.tensor_add(
                            out=st[:, 0:512],
                            in0=st[:, 0:512].bitcast(F32),
                            in1=sp[0:1, :])
                        nc.vector.tensor_add(
                            out=st[:, 512:1024],
                            in0=st[:, 512:1024].bitcast(F32),
                            in1=sq[0:1, :])

                def stats(qn):
                    """bounce accumulated stats to DRAM + AllGather."""
                    (st,) = stats_ctx.pop(qn)
                    nc.sync.dma_start(
                        out=bass.AP(tensor=cc_in.tensor,
                                    offset=cc_in.offset + qn * 1024,
                                    ap=[[1, 1024]]),
                        in_=st)
                    nc.gpsimd.collective_compute(
                        "AllGather", ALU.bypass,
                        replica_groups=REPLICA_GROUPS,
                        ins=[cc_in[qn].opt()], outs=[cc_out[qn].opt()],
                    )

                def apply_ln(qn):
                    """Reduce gathered stats (matmul), row math on [1,512],
                    broadcast gamma*A / gamma*B+beta rows (matmuls), apply,
                    DMA out."""
                    qs = slice(qn * 512, (qn + 1) * 512)
                    cc_sb = stg.tile([4, 1024], F32R, name="ccsb")
                    nc.sync.dma_start(out=cc_sb, in_=cc_out[qn].rearrange(
                        "c v s -> c (v s)"))
                    stt = pps.tile([128, 512], F32, name="gps")
                    stq = pps.tile([128, 512], F32, name="gps")
                    nc.tensor.matmul(
                        out=stt[0:1, :], lhsT=oD,
                        rhs=cc_sb[:, 0:512])
                    nc.tensor.matmul(
                        out=stq[0:1, :], lhsT=oD,
                        rhs=cc_sb[:, 512:1024])
                    # row math on [1, 512]: stt[0]=mu, stt[32]=E[x^2];
                    # rm is reused in place down the chain
                    rm = small.tile([1, 512], F32, name="rm")
                    nc.scalar.activation(
                        out=rm, in_=stt[0:1, :], func=AF.Square)
                    nc.vector.scalar_tensor_tensor(
                        out=rm, in0=stq[0:1, :], scalar=1.0,
                        in1=rm, op0=ALU.mult, op1=ALU.subtract)
                    nc.scalar.activation(
                        out=rm, in_=rm, func=AF.Sqrt, bias=eps_s[0:1, :])
                    arow = small.tile([1, 512], F32R, name="arow")
                    nc.vector.reciprocal(out=arow, in_=rm)
                    # brow = -mu * rstd
                    nc.vector.scalar_tensor_tensor(
                        out=brow, in0=stt[0:1, :], scalar=-1.0,
                        in1=arow.bitcast(F32), op0=ALU.mult, op1=ALU.mult)
                    # broadcast rows with gamma/beta folded in:
                    #   ab[:,0:512] = gamma[p]*A[s]; ab[:,512:]=gamma*B+beta
                    ot = stg.tile([128, MT, 512], BF16, name="ot")
                    for t in range(MT):
                        abA = pps.tile([128, 512], F32, name="gps")
                        abB = pps.tile([128, 512], F32, name="gps")
                        nc.tensor.matmul(
                            out=abA, lhsT=gmr[:, t, :], rhs=arow)
                        nc.tensor.matmul(
                            out=abB, lhsT=gmr[:, t, :], rhs=brow)
                        ota = stg.tile([128, 512], F32, name="tmp")
                        nc.vector.tensor_mul(
                            out=ota, in0=x[:, t, qs], in1=abA)
                        # ot = (ota + beta) + gamma*brow_bcast
                        nc.vector.scalar_tensor_tensor(
                            out=ot[:, t, :], in0=ota,
                            scalar=bt_s[:, t:t + 1], in1=abB,
                            op0=ALU.add, op1=ALU.add)
                    out_t = out_d.rearrange("(t p) s -> p t s", p=128)
                    nc.sync.dma_start(out=out_t[:, :, qs], in_=ot)

                # ================= main loop =================
                for qn in range(NS):
                    for hp in range(MT):
                        attention_block(hp, qn)
                        stats_part(qn, hp)
                    stats(qn)
                    if qn == 1:
                        apply_ln(0)
                # applies 1 and 2 run inside AllGather(3)'s flight window
                apply_ln(1)
                apply_ln(2)
                apply_ln(NS - 1)
            hsbp_cm.__exit__(None, None, None)
            p1sb_cm.__exit__(None, None, None)
    lp_cm.__exit__(None, None, None)
    _split_waits(nc)
    return nc


_NC = None
LAST_RESULT = None


def _get_nc():
    global _NC
    if _NC is None:
        _NC = build_bass()
    return _NC


def kernel(hidden_states, Wq, bq, Wk, bk, Wv, bv, Wp, gamma, beta):
    hs = np.ascontiguousarray(np.asarray(hidden_states, dtype=np.float32))
    Wq = np.asarray(Wq, np.float32)
    Wk = np.asarray(Wk, np.float32)
    Wv = np.asarray(Wv, np.float32)
    Wp = np.asarray(Wp, np.float32)
    bq = np.asarray(bq, np.float32)
    bk = np.asarray(bk, np.float32)
    bv = np.asarray(bv, np.float32)
    gamma = np.asarray(gamma, np.float32)
    beta = np.asarray(beta, np.float32)

    nc = _get_nc()
    in_maps = []
    for c in range(NCORES):
        b, g = divmod(c, GROUPS)
        sl = slice(g * DC, (g + 1) * DC)
        gb = np.stack([gamma[sl].reshape(MT, 128),
                       beta[sl].reshape(MT, 128)])  # [2, MT, 128]
        in_maps.append({
            "hsT": np.ascontiguousarray(hs[b].T).astype(
                ml_dtypes.bfloat16),
            "wqT": np.ascontiguousarray(Wq[sl].T).astype(ml_dtypes.bfloat16),
            "wkT": np.ascontiguousarray(Wk[sl].T).astype(ml_dtypes.bfloat16),
            "wvT": np.ascontiguousarray(Wv[sl].T).astype(ml_dtypes.bfloat16),
            "wpT": np.ascontiguousarray(Wp[sl].T).astype(ml_dtypes.bfloat16),
            "bq": np.ascontiguousarray(bq[sl]),
            "bk": np.ascontiguousarray(bk[sl]),
            "bv": np.ascontiguousarray(bv[sl]).astype(ml_dtypes.bfloat16),
            "gbrows": np.ascontiguousarray(gb),
        })
    res = run_bass_kernel_spmd(nc, in_maps, core_ids=list(range(NCORES)))
    global LAST_RESULT
    LAST_RESULT = res
    out = np.empty((B, S, D), np.float32)
    for c, r in enumerate(res.results):
        b, g = divmod(c, GROUPS)
        out[b, :, g * DC:(g + 1) * DC] = np.asarray(
            r["outT"]).astype(np.float32).T
    return out


# revision 49
# speedup vs baseline: 1.1940x; 1.0121x over previous
"""Trainium2 Bass kernel for nn_BertAttention_78554951843978.

Reference computation (B=2, S=2048, D=1024, H=16, hd=64, fp32):
    q = split_heads(hs @ Wq.T + bq); k = ...; v = ...
    probs = softmax(q k^T / sqrt(64)); ctx = probs @ v
    x = relu(merge_heads(ctx) + hs @ Wp.T)
    out = layernorm(x) * gamma + beta        (eps = 1e-12)

Sharding (8 cores): data-parallel over B (2 groups of 4 cores), tensor-
parallel over heads within a group (4 heads / 256 dims of D per core).

Structure (v3 — overlap-optimized):
  - hsT DMA'd in S-chunks so the first score matmul fires early
  - all matmul operands fp32r (bf16 stationaries cost a separate
    Ldweights instruction on the saturated PE sequencer)
  - attention loop is qn-major (query chunk outer, head-pair inner) so
    layernorm stats for each chunk complete early
  - per-chunk stats AllGather (4 small AGs, pipelined under compute)
    + matmul-based local reduce replaces the terminal AllReduce
  - all partition broadcasts (1/denom, LN scale/shift rows) are K=1/K=2
    ones-matmuls on the PE instead of DRAM DMA bounces
  - per-chunk LN apply + bf16 output DMA, emitted one chunk behind
"""

import numpy as np
import ml_dtypes

import concourse.bass as bass
import concourse.tile as tile
from concourse import mybir
from concourse.bass_utils import run_bass_kernel_spmd

B, S, D, H = 2, 2048, 1024, 16
HD = 64
NCORES = 8
GROUPS = 4          # cores per batch
DC = D // GROUPS    # 256 dims per core
EPS = 1e-12

F32 = mybir.dt.float32
F32R = mybir.dt.float32r
BF16 = mybir.dt.bfloat16
FP8 = mybir.dt.float8e4
VW = 68   # padded head width in vA8 (272B parity stride, 16B-aligned)
AF = mybir.ActivationFunctionType
ALU = mybir.AluOpType

KT = D // 128    # 8 contraction tiles
MT = DC // 128   # 2 output tiles of 128 dims (a head pair each)
NS = S // 512    # 4 query chunks of 512
ST = S // 128    # 16 key tiles of 128

REPLICA_GROUPS = [[0, 1, 2, 3], [4, 5, 6, 7]]


def _split_waits(nc, keep=1):
    """This container's walrus rejects >1 sem wait per (non-EVSEM)
    instruction ("Too many sync wait commands"); hoist extras onto
    preceding single-wait NOPs on the same engine."""
    for bb in nc.main_func.blocks:
        insts = list(bb.instructions)
        out_list = []
        changed = False
        for inst in insts:
            si = inst.sync_info
            cap = 2 if isinstance(inst, mybir.InstEventSemaphore) else keep
            if si is not None and si.on_wait is not None and len(si.on_wait) > cap:
                waits = list(si.on_wait)
                for w in waits[cap:]:
                    out_list.append(mybir.InstNoOp(
                        name=nc.get_next_instruction_name(),
                        engine=inst.engine,
                        ins=[], outs=[],
                        sync_info=mybir.SyncInfo(on_wait=[w], on_update=[]),
                        bass_nofuse=True,
                    ))
                inst.sync_info = mybir.SyncInfo(
                    on_wait=waits[:cap], on_update=list(si.on_update or []))
                changed = True
            out_list.append(inst)
        if changed:
            bb.instructions = out_list


def build_bass():
    nc = bass.Bass(num_devices=NCORES)

    # ---------------- DRAM I/O ----------------
    hsT_d = nc.dram_tensor("hsT", [D, S], BF16, kind="ExternalInput")
    wqT_d = nc.dram_tensor("wqT", [D, DC], BF16, kind="ExternalInput")
    wkT_d = nc.dram_tensor("wkT", [D, DC], BF16, kind="ExternalInput")
    wvT_d = nc.dram_tensor("wvT", [D, DC], BF16, kind="ExternalInput")
    wpT_d = nc.dram_tensor("wpT", [D, DC], BF16, kind="ExternalInput")
    bq_d = nc.dram_tensor("bq", [DC], F32, kind="ExternalInput")
    bk_d = nc.dram_tensor("bk", [DC], F32, kind="ExternalInput")
    bvh_d = nc.dram_tensor("bv", [DC], BF16, kind="ExternalInput")
    gbr_d = nc.dram_tensor("gbrows", [2, MT, 128], F32R,
                           kind="ExternalInput")
    out_d = nc.dram_tensor("outT", [DC, S], BF16, kind="ExternalOutput")

    lp_cm = nc.allow_low_precision(reason="rel-err budget 2e-2; bf16 ok")
    lp_cm.__enter__()
    with tile.TileContext(nc) as tc:
        with (
            tc.tile_pool(name="persist", bufs=1) as persist,
            tc.tile_pool(name="dram", bufs=1, space="DRAM") as dram,
        ):
            # ------------- persistent SBUF -------------
            qT = persist.tile([128, MT, S], F32R)
            kT = persist.tile([128, MT, S], F32R)
            x = persist.tile([128, MT, S], F32)     # res, x, then relu(x)
            # aug V: [p, s-tile, head, dim|ones]
            vA = persist.tile([128, ST, GROUPS, HD + 1], F32R)
            onesc = persist.tile([128, 1], F32R)             # stats lhsT
            # small constants: cols = bq(2)|bk(2)|eps(1)|beta(2)|1.0|1/D
            cst = persist.tile([128, 9], F32)
            bq_s, bk_s = cst[:, 0:2], cst[:, 2:4]
            eps_s = cst[:, 4:5]
            bt_s = cst[:, 5:7]
            bv_b = persist.tile([128, DC], BF16)             # bv bcast
            gmr = persist.tile([1, MT, 128], F32R)           # gamma row
            oD = persist.tile([4, 1], F32R)                  # 1/D lhsT
            brow = persist.tile([1, 512], F32R)              # -mu*rstd row

            # DRAM scratch
            scr = dram.tile([MT * NS, 2, 512], F32)     # denom bounce
            cc_in = dram.tile([NS, 2, 512], F32R)
            cc_out = dram.tile([NS, GROUPS, 2, 512], F32R)

            p1sb_cm = tc.tile_pool(name="p1sb", bufs=1)
            p1sb = p1sb_cm.__enter__()
            hsT = p1sb.tile([128, KT, S], F32R)
            wq = p1sb.tile([128, KT, MT, 128], F32R)
            wk = p1sb.tile([128, KT, MT, 128], F32R)
            wv = p1sb.tile([128, KT, DC], F32R)
            wp = p1sb.tile([128, KT, MT, 128], F32R)

            # ---------------- input DMAs ----------------
            # Bulk tensors on the SP queue, ordered so the first score
            # matmul's deps land first. hsT is chunked along S.
            hsT_t = hsT_d.rearrange("(t p) s -> p t s", p=128)
            wq_t = wqT_d.rearrange("(t p) (m f) -> p t m f", p=128, f=128)
            wk_t = wkT_d.rearrange("(t p) (m f) -> p t m f", p=128, f=128)
            wv_t = wvT_d.rearrange("(t p) c -> p t c", p=128)
            wp_t = wpT_d.rearrange("(t p) (m f) -> p t m f", p=128, f=128)

            hsbp_cm = tc.tile_pool(name="hsbp", bufs=2)
            hsbp = hsbp_cm.__enter__()

            def wload(w_sb, src_ap, wb, eng="v"):
                nc.sync.dma_start(out=wb, in_=src_ap)
                # upcast bf16 -> fp32r; the startup chain is split across
                # DVE and the (still idle) ACT engine
                if eng == "a":
                    nc.scalar.activation(out=w_sb, in_=wb, func=AF.Copy)
                else:
                    nc.vector.tensor_scalar_mul(
                        out=w_sb, in0=wb, scalar1=1.0)

            CW = 128

            def hs_chunk(sn, eng="v"):
                sl2 = slice(sn * CW, (sn + 1) * CW)
                hsb = hsbp.tile([128, KT, CW], BF16, name="hsb")
                nc.sync.dma_start(out=hsb, in_=hsT_t[:, :, sl2])
                wload_cast(hsT[:, :, sl2], hsb, eng)

            def wload_cast(dst, srcb, eng):
                if eng == "a":
                    nc.scalar.activation(out=dst, in_=srcb, func=AF.Copy)
                else:
                    nc.vector.tensor_scalar_mul(
                        out=dst, in0=srcb, scalar1=1.0)

            wload(wk[:, :, 0, :], wk_t[:, :, 0, :],
                  hsbp.tile([128, KT, CW], BF16, name="hsb"), "v")
            wload(wq[:, :, 0, :], wq_t[:, :, 0, :],
                  hsbp.tile([128, KT, CW], BF16, name="hsb"), "a")
            for sn in range(4):
                hs_chunk(sn, "va"[sn % 2])
            wload(wv[:, :, 0:128], wv_t[:, :, 0:128],
                  hsbp.tile([128, KT, CW], BF16, name="hsb"), "v")
            wload(wv[:, :, 128:256], wv_t[:, :, 128:256],
                  hsbp.tile([128, KT, CW], BF16, name="hsb"), "a")

            def load_rest():
                for sn in range(4, 16):
                    hs_chunk(sn)
                wload(wp[:, :, 0, :], wp_t[:, :, 0, :],
                      hsbp.tile([128, KT, CW], BF16, name="hsb"))
                wload(wp[:, :, 1, :], wp_t[:, :, 1, :],
                      hsbp.tile([128, KT, CW], BF16, name="hsb"))
                wload(wk[:, :, 1, :], wk_t[:, :, 1, :],
                      hsbp.tile([128, KT, CW], BF16, name="hsb"))
                wload(wq[:, :, 1, :], wq_t[:, :, 1, :],
                      hsbp.tile([128, KT, CW], BF16, name="hsb"))

            # small constants on the gpsimd (SWDGE) queue
            nc.gpsimd.dma_start(
                out=bq_s, in_=bq_d.rearrange("(m p) -> p m", p=128))
            nc.gpsimd.dma_start(
                out=bk_s, in_=bk_d.rearrange("(m p) -> p m", p=128))
            nc.gpsimd.dma_start(out=gmr, in_=gbr_d[0:1, :, :])
            nc.gpsimd.dma_start(
                out=bt_s,
                in_=gbr_d[1:2, :, :].bitcast(F32).rearrange(
                    "r m p -> (r p) m"))
            nc.gpsimd.dma_start(out=bv_b, in_=bass.AP(
                tensor=bvh_d[:].tensor, offset=0, ap=[[0, 128], [1, DC]]))
            nc.vector.memset(eps_s, EPS)
            nc.vector.memset(cst[:, 7:8], 1.0)
            nc.vector.memset(cst[:, 8:9], 1.0 / D)
            # f32 -> f32r rounding casts via SWDGE dma (engine memset to an
            # f32r tile fails BIR verification)
            nc.gpsimd.dma_start(out=oD, in_=cst[0:4, 8:9])
            nc.gpsimd.dma_start(out=onesc, in_=cst[:, 7:8])
            nc.vector.memset(vA[:, :, :, HD:HD + 1].bitcast(F32), 1.0)
            onesr = onesc

            with (
                tc.tile_pool(name="pps", bufs=2, space="PSUM") as pps,
                tc.tile_pool(name="scps", bufs=2, space="PSUM") as scps,
                tc.tile_pool(name="ctxps", bufs=2, space="PSUM") as ctxps,
                tc.tile_pool(name="ptp", bufs=2) as ptp,
                tc.tile_pool(name="small", bufs=1) as small,
                tc.tile_pool(name="stg", bufs=1) as stg,
                tc.tile_pool(name="x2p", bufs=1) as x2p,
            ):
                def proj_group(w_sb, m, n, bias, out_sb):
                    """One [128,512] output block of a W-stationary proj."""
                    ps = pps.tile([128, 512], F32, name="gps")
                    for k in range(KT):
                        nc.tensor.matmul(
                            out=ps, lhsT=w_sb[:, k, m, :],
                            rhs=hsT[:, k, n * 512:(n + 1) * 512],
                            start=(k == 0), stop=(k == KT - 1))
                    o = out_sb[:, m, n * 512:(n + 1) * 512]
                    if bias is not None:
                        nc.vector.tensor_scalar_add(out=o, in0=ps, scalar1=bias)
                    else:
                        # x feeds an fp32r matmul: every write into x must
                        # carry an fp32r output dtype for BIR verification
                        nc.vector.tensor_scalar_add(
                            out=o.bitcast(F32R), in0=ps, scalar1=0.0)

                def v_group(j):
                    """V (natural layout) for s-tile j, hs stationary."""
                    ps = pps.tile([128, 512], F32, name="gps")
                    for k in range(KT):
                        nc.tensor.matmul(
                            out=ps[:, 0:DC],
                            lhsT=hsT[:, k, j * 128:(j + 1) * 128],
                            rhs=wv[:, k, :],
                            start=(k == 0), stop=(k == KT - 1))
                    nc.vector.tensor_add(
                        out=vA[:, j, :, 0:HD],
                        in0=ps[:, 0:DC].rearrange("p (h d) -> p h d", d=HD),
                        in1=bv_b.rearrange("p (h d) -> p h d", d=HD))

                def g_q(m, n):
                    return lambda: proj_group(wq, m, n, bq_s[:, m:m + 1], qT)

                def g_k(m, n):
                    return lambda: proj_group(wk, m, n, bk_s[:, m:m + 1], kT)

                def g_r(m, n):
                    return lambda: proj_group(wp, m, n, None, x)

                def g_v(j):
                    return lambda: v_group(j)

                # upfront: only what the first score matmul needs
                g_q(0, 0)()
                g_k(0, 0)()
                load_rest()

                # filler schedule per (qn, hp) block
                fillers = {
                    (0, 0): [g_v(0), g_v(1), g_v(2), g_v(3), g_k(0, 1),
                             g_v(4), g_v(5), g_v(6), g_v(7), g_k(0, 2),
                             g_v(8), g_v(9), g_v(10), g_v(11), g_k(0, 3),
                             g_v(12), g_v(13), g_v(14), g_v(15), g_r(0, 0),
                             g_q(1, 0), g_k(1, 0)],
                    (0, 1): [g_k(1, 1), g_k(1, 2), g_k(1, 3), g_r(1, 0),
                             g_q(0, 1), g_r(0, 1)],
                    (1, 0): [g_q(1, 1), g_r(1, 1)],
                    (1, 1): [g_q(0, 2), g_r(0, 2)],
                    (2, 0): [g_q(1, 2), g_r(1, 2)],
                    (2, 1): [g_q(0, 3), g_r(0, 3)],
                    (3, 0): [g_q(1, 3), g_r(1, 3)],
                    (3, 1): [],
                }

                def division(hp, qn, ctx0, ctx1):
                    """x[:, hp, qs] += ctx/denom (x holds res).

                    1/denom rows are broadcast down the partitions with a
                    K=1 ones-matmul instead of a DRAM DMA bounce."""
                    qs = slice(qn * 512, (qn + 1) * 512)
                    blk = qn * MT + hp
                    ctxc = stg.tile([128, 1024], F32, name="ctxc")
                    nc.vector.tensor_copy(
                        out=ctxc[0:HD + 1, 0:512], in_=ctx0[0:HD + 1, :])
                    nc.vector.tensor_copy(
                        out=ctxc[0:HD + 1, 512:1024], in_=ctx1[0:HD + 1, :])
                    rr = small.tile([1, 1024], F32, name="rr")
                    nc.vector.reciprocal(
                        out=rr[:, 0:512], in_=ctxc[HD:HD + 1, 0:512])
                    nc.vector.reciprocal(
                        out=rr[:, 512:1024], in_=ctxc[HD:HD + 1, 512:1024])
                    nc.sync.dma_start(
                        out=bass.AP(tensor=scr.tensor,
                                    offset=scr.offset + blk * 1024,
                                    ap=[[1, 1024]]),
                        in_=rr)
                    rbs = stg.tile([64, 1024], F32, name="rbs")
                    nc.sync.dma_start(
                        out=rbs,
                        in_=bass.AP(tensor=scr.tensor,
                                    offset=scr.offset + blk * 1024,
                                    ap=[[0, 64], [512, 2], [1, 512]]))
                    tmp = stg.tile([128, 512], F32, name="tmp")
                    nc.vector.tensor_mul(
                        out=tmp[0:64, :], in0=ctxc[0:HD, 0:512],
                        in1=rbs[:, 0:512])
                    nc.vector.tensor_mul(
                        out=tmp[64:128, :], in0=ctxc[0:HD, 512:1024],
                        in1=rbs[:, 512:1024])
                    nc.vector.tensor_add(
                        out=x[:, hp, qs].bitcast(F32R), in0=x[:, hp, qs],
                        in1=tmp)

                def attention_block(hp, qn):
                    qs = slice(qn * 512, (qn + 1) * 512)
                    ctx0 = ctxps.tile([128, 512], F32, name="ctx")
                    ctx1 = ctxps.tile([128, 512], F32, name="ctx")
                    fl = list(fillers[(qn, hp)])
                    # spread fillers evenly over the 16 ks slots
                    per_slot = [0] * ST
                    for i in range(len(fl)):
                        per_slot[(i * ST) // max(1, len(fl))] += 1
                    fl.reverse()

                    def ctx_mms(pt, ks):
                        nc.tensor.matmul(
                            out=ctx0[0:HD + 1, :],
                            lhsT=vA[:, ks, 2 * hp, :],
                            rhs=pt[:, 0:512],
                            start=(ks == 0), stop=(ks == ST - 1))
                        nc.tensor.matmul(
                            out=ctx1[0:HD + 1, :],
                            lhsT=vA[:, ks, 2 * hp + 1, :],
                            rhs=pt[:, 512:1024],
                            start=(ks == 0), stop=(ks == ST - 1))

                    prev = None
                    for ks in range(ST):
                        sc = scps.tile([128, 1024], F32, name="sc")
                        kslc = slice(ks * 128, (ks + 1) * 128)
                        nc.tensor.matmul(
                            out=sc[:, 0:512],
                            lhsT=kT[0:64, hp, kslc],
                            rhs=qT[0:64, hp, qs])
                        nc.tensor.matmul(
                            out=sc[:, 512:1024],
                            lhsT=kT[64:128, hp, kslc],
                            rhs=qT[64:128, hp, qs])
                        pt = ptp.tile([128, 1024], F32R, name="pt")
                        nc.scalar.activation(
                            out=pt, in_=sc, func=AF.Exp,
                            scale=float(1.0 / np.sqrt(HD)))
                        for _ in range(per_slot[ks]):
                            if fl:
                                fl.pop()()
                        if prev is not None:
                            ctx_mms(*prev)
                        prev = (pt, ks)
                    ctx_mms(*prev)
                    division(hp, qn, ctx0, ctx1)

                stats_ctx = {}

                def stats_part(qn, t):
                    """relu + square + stats-matmul contribution of head
                    pair t, emitted right after division(t, qn). The psum
                    rows are merged into the SBUF accumulator immediately
                    so the psum pool is never held across a block."""
                    qs = slice(qn * 512, (qn + 1) * 512)
                    if t == 0:
                        stats_ctx[qn] = (
                            small.tile([1, 1024], F32R, name="st"),)
                    (st,) = stats_ctx[qn]
                    x2 = x2p.tile([128, 512], F32R, name="x2")
                    nc.vector.tensor_scalar_max(
                        out=x[:, t, qs].bitcast(F32R), in0=x[:, t, qs],
                        scalar1=0.0)
                    nc.scalar.activation(
                        out=x2, in_=x[:, t, qs], func=AF.Square)
                    sp = pps.tile([128, 512], F32, name="gps")
                    sq = pps.tile([128, 512], F32, name="gps")
                    nc.tensor.matmul(
                        out=sp[0:1, :], lhsT=onesr,
                        rhs=x[:, t, qs].bitcast(F32R))
                    nc.tensor.matmul(
                        out=sq[0:1, :], lhsT=onesr, rhs=x2)
                    if t == 0:
                        nc.vector.tensor_scalar_mul(
                            out=st[:, 0:512], in0=sp[0:1, :], scalar1=1.0)
                        nc.vector.tensor_scalar_mul(
                            out=st[:, 512:1024], in0=sq[0:1, :],
                            scalar1=1.0)
                    else:
                        nc.vector.tensor_add(
                            out=st[:, 0:512],
                            in0=st[:, 0:512].bitcast(F32),
                            in1=sp[0:1, :])
                        nc.vector.tensor_add(
                            out=st[:, 512:1024],
                            in0=st[:, 512:1024].bitcast(F32),
                            in1=sq[0:1, :])

                def stats(qn):
                    """bounce accumulated stats to DRAM + AllGather."""
                    (st,) = stats_ctx.pop(qn)
                    nc.sync.dma_start(
                        out=bass.AP(tensor=cc_in.tensor,
                                    offset=cc_in.offset + qn * 1024,
                                    ap=[[1, 1024]]),
                        in_=st)
                    nc.gpsimd.collective_compute(
                        "AllGather", ALU.bypass,
                        replica_groups=REPLICA_GROUPS,
                        ins=[cc_in[qn].opt()], outs=[cc_out[qn].opt()],
                    )

                def apply_ln(qn):
                    """Reduce gathered stats (matmul), row math on [1,512],
                    broadcast gamma*A / gamma*B+beta rows (matmuls), apply,
                    DMA out."""
                    qs = slice(qn * 512, (qn + 1) * 512)
                    cc_sb = stg.tile([4, 1024], F32R, name="ccsb")
                    nc.sync.dma_start(out=cc_sb, in_=cc_out[qn].rearrange(
                        "c v s -> c (v s)"))
                    stt = pps.tile([128, 512], F32, name="gps")
                    stq = pps.tile([128, 512], F32, name="gps")
                    nc.tensor.matmul(
                        out=stt[0:1, :], lhsT=oD,
                        rhs=cc_sb[:, 0:512])
                    nc.tensor.matmul(
                        out=stq[0:1, :], lhsT=oD,
                        rhs=cc_sb[:, 512:1024])
                    # row math on [1, 512]: stt[0]=mu, stt[32]=E[x^2];
                    # rm is reused in place down the chain
                    rm = small.tile([1, 512], F32, name="rm")
                    nc.scalar.activation(
                        out=rm, in_=stt[0:1, :], func=AF.Square)
                    nc.vector.scalar_tensor_tensor(
                        out=rm, in0=stq[0:1, :], scalar=1.0,
                        in1=rm, op0=ALU.mult, op1=ALU.subtract)
                    nc.scalar.activation(
                        out=rm, in_=rm, func=AF.Sqrt, bias=eps_s[0:1, :])
                    arow = small.tile([1, 512], F32R, name="arow")
                    nc.vector.reciprocal(out=arow, in_=rm)
                    # brow = -mu * rstd
                    nc.vector.scalar_tensor_tensor(
                        out=brow, in0=stt[0:1, :], scalar=-1.0,
                        in1=arow.bitcast(F32), op0=ALU.mult, op1=ALU.mult)
                    # broadcast rows with gamma/beta folded in:
                    #   ab[:,0:512] = gamma[p]*A[s]; ab[:,512:]=gamma*B+beta
                    ot = stg.tile([128, MT, 512], BF16, name="ot")
                    for t in range(MT):
                        abA = pps.tile([128, 512], F32, name="gps")
                        abB = pps.tile([128, 512], F32, name="gps")
                        nc.tensor.matmul(
                            out=abA, lhsT=gmr[:, t, :], rhs=arow)
                        nc.tensor.matmul(
                            out=abB, lhsT=gmr[:, t, :], rhs=brow)
                        ota = stg.tile([128, 512], F32, name="tmp")
                        nc.vector.tensor_mul(
                            out=ota, in0=x[:, t, qs], in1=abA)
                        # ot = (ota + beta) + gamma*brow_bcast
                        nc.vector.scalar_tensor_tensor(
                            out=ot[:, t, :], in0=ota,
                            scalar=bt_s[:, t:t + 1], in1=abB,
                            op0=ALU.add, op1=ALU.add)
                    out_t = out_d.rearrange("(t p) s -> p t s", p=128)
                    nc.sync.dma_start(out=out_t[:, :, qs], in_=ot)

                # ================= main loop =================
                for qn in range(NS):
                    for hp in range(MT):
                        attention_block(hp, qn)
                        stats_part(qn, hp)
                    stats(qn)
                    if qn == 1:
                        apply_ln(0)
                # applies 1 and 2 run inside AllGather(3)'s flight window
                apply_ln(1)
                apply_ln(2)
                apply_ln(NS - 1)
            hsbp_cm.__exit__(None, None, None)
            p1sb_cm.__exit__(None, None, None)
    lp_cm.__exit__(None, None, None)
    _split_waits(nc)
    return nc


_NC = None
LAST_RESULT = None


def _get_nc():
    global _NC
    if _NC is None:
        _NC = build_bass()
    return _NC


def kernel(hidden_states, Wq, bq, Wk, bk, Wv, bv, Wp, gamma, beta):
    hs = np.ascontiguousarray(np.asarray(hidden_states, dtype=np.float32))
    Wq = np.asarray(Wq, np.float32)
    Wk = np.asarray(Wk, np.float32)
    Wv = np.asarray(Wv, np.float32)
    Wp = np.asarray(Wp, np.float32)
    bq = np.asarray(bq, np.float32)
    bk = np.asarray(bk, np.float32)
    bv = np.asarray(bv, np.float32)
    gamma = np.asarray(gamma, np.float32)
    beta = np.asarray(beta, np.float32)

    nc = _get_nc()
    in_maps = []
    for c in range(NCORES):
        b, g = divmod(c, GROUPS)
        sl = slice(g * DC, (g + 1) * DC)
        gb = np.stack([gamma[sl].reshape(MT, 128),
                       beta[sl].reshape(MT, 128)])  # [2, MT, 128]
        in_maps.append({
            "hsT": np.ascontiguousarray(hs[b].T).astype(
                ml_dtypes.bfloat16),
            "wqT": np.ascontiguousarray(Wq[sl].T).astype(ml_dtypes.bfloat16),
            "wkT": np.ascontiguousarray(Wk[sl].T).astype(ml_dtypes.bfloat16),
            "wvT": np.ascontiguousarray(Wv[sl].T).astype(ml_dtypes.bfloat16),
            "wpT": np.ascontiguousarray(Wp[sl].T).astype(ml_dtypes.bfloat16),
            "bq": np.ascontiguousarray(bq[sl]),
            "bk": np.ascontiguousarray(bk[sl]),
            "bv": np.ascontiguousarray(bv[sl]).astype(ml_dtypes.bfloat16),
            "gbrows": np.ascontiguousarray(gb),
        })
    res = run_bass_kernel_spmd(nc, in_maps, core_ids=list(range(NCORES)))
    global LAST_RESULT
    LAST_RESULT = res
    out = np.empty((B, S, D), np.float32)
    for c, r in enumerate(res.results):
        b, g = divmod(c, GROUPS)
        out[b, :, g * DC:(g + 1) * DC] = np.asarray(
            r["outT"]).astype(np.float32).T
    return out


# revision 56
# speedup vs baseline: 1.1997x; 1.0048x over previous
"""Trainium2 Bass kernel for nn_BertAttention_78554951843978.

Reference computation (B=2, S=2048, D=1024, H=16, hd=64, fp32):
    q = split_heads(hs @ Wq.T + bq); k = ...; v = ...
    probs = softmax(q k^T / sqrt(64)); ctx = probs @ v
    x = relu(merge_heads(ctx) + hs @ Wp.T)
    out = layernorm(x) * gamma + beta        (eps = 1e-12)

Sharding (8 cores): data-parallel over B (2 groups of 4 cores), tensor-
parallel over heads within a group (4 heads / 256 dims of D per core).

Structure (v3 — overlap-optimized):
  - hsT DMA'd in S-chunks so the first score matmul fires early
  - all matmul operands fp32r (bf16 stationaries cost a separate
    Ldweights instruction on the saturated PE sequencer)
  - attention loop is qn-major (query chunk outer, head-pair inner) so
    layernorm stats for each chunk complete early
  - per-chunk stats AllGather (4 small AGs, pipelined under compute)
    + matmul-based local reduce replaces the terminal AllReduce
  - all partition broadcasts (1/denom, LN scale/shift rows) are K=1/K=2
    ones-matmuls on the PE instead of DRAM DMA bounces
  - per-chunk LN apply + bf16 output DMA, emitted one chunk behind
"""

import numpy as np
import ml_dtypes

import concourse.bass as bass
import concourse.tile as tile
from concourse import mybir
from concourse.bass_utils import run_bass_kernel_spmd

B, S, D, H = 2, 2048, 1024, 16
HD = 64
NCORES = 8
GROUPS = 4          # cores per batch
DC = D // GROUPS    # 256 dims per core
EPS = 1e-12

F32 = mybir.dt.float32
F32R = mybir.dt.float32r
BF16 = mybir.dt.bfloat16
FP8 = mybir.dt.float8e4
VW = 68   # padded head width in vA8 (272B parity stride, 16B-aligned)
AF = mybir.ActivationFunctionType
ALU = mybir.AluOpType

KT = D // 128    # 8 contraction tiles
MT = DC // 128   # 2 output tiles of 128 dims (a head pair each)
NS = S // 512    # 4 query chunks of 512
ST = S // 128    # 16 key tiles of 128

REPLICA_GROUPS = [[0, 1, 2, 3], [4, 5, 6, 7]]


def _split_waits(nc, keep=1):
    """This container's walrus rejects >1 sem wait per (non-EVSEM)
    instruction ("Too many sync wait commands"); hoist extras onto
    preceding single-wait NOPs on the same engine."""
    for bb in nc.main_func.blocks:
        insts = list(bb.instructions)
        out_list = []
        changed = False
        for inst in insts:
            si = inst.sync_info
            cap = 2 if isinstance(inst, mybir.InstEventSemaphore) else keep
            if si is not None and si.on_wait is not None and len(si.on_wait) > cap:
                waits = list(si.on_wait)
                for w in waits[cap:]:
                    out_list.append(mybir.InstNoOp(
                        name=nc.get_next_instruction_name(),
                        engine=inst.engine,
                        ins=[], outs=[],
                        sync_info=mybir.SyncInfo(on_wait=[w], on_update=[]),
                        bass_nofuse=True,
                    ))
                inst.sync_info = mybir.SyncInfo(
                    on_wait=waits[:cap], on_update=list(si.on_update or []))
                changed = True
            out_list.append(inst)
        if changed:
            bb.instructions = out_list


def build_bass():
    nc = bass.Bass(num_devices=NCORES)

    # ---------------- DRAM I/O ----------------
    hsT_d = nc.dram_tensor("hsT", [D, S], BF16, kind="ExternalInput")
    wqT_d = nc.dram_tensor("wqT", [D, DC], BF16, kind="ExternalInput")
    wkT_d = nc.dram_tensor("wkT", [D, DC], BF16, kind="ExternalInput")
    wvT_d = nc.dram_tensor("wvT", [D, DC], BF16, kind="ExternalInput")
    wpT_d = nc.dram_tensor("wpT", [D, DC], BF16, kind="ExternalInput")
    bq_d = nc.dram_tensor("bq", [DC], F32, kind="ExternalInput")
    bk_d = nc.dram_tensor("bk", [DC], F32, kind="ExternalInput")
    bvh_d = nc.dram_tensor("bv", [DC], BF16, kind="ExternalInput")
    gbr_d = nc.dram_tensor("gbrows", [2, MT, 128], F32R,
                           kind="ExternalInput")
    out_d = nc.dram_tensor("outT", [DC, S], BF16, kind="ExternalOutput")

    lp_cm = nc.allow_low_precision(reason="rel-err budget 2e-2; bf16 ok")
    lp_cm.__enter__()
    with tile.TileContext(nc) as tc:
        with (
            tc.tile_pool(name="persist", bufs=1) as persist,
            tc.tile_pool(name="dram", bufs=1, space="DRAM") as dram,
        ):
            # ------------- persistent SBUF -------------
            qT = persist.tile([128, MT, S], F32R)
            kT = persist.tile([128, MT, S], F32R)
            x = persist.tile([128, MT, S], F32)     # res, x, then relu(x)
            # aug V: [p, s-tile, head, dim|ones]
            vA = persist.tile([128, ST, GROUPS, HD + 1], F32R)
            onesc = persist.tile([128, 1], F32R)             # stats lhsT
            # small constants: cols = bq(2)|bk(2)|eps(1)|beta(2)|1.0|1/D
            cst = persist.tile([128, 9], F32)
            bq_s, bk_s = cst[:, 0:2], cst[:, 2:4]
            eps_s = cst[:, 4:5]
            bt_s = cst[:, 5:7]
            bv_b = persist.tile([128, DC], BF16)             # bv bcast
            gmr = persist.tile([1, MT, 128], F32R)           # gamma row
            oD = persist.tile([4, 1], F32R)                  # 1/D lhsT
            brow = persist.tile([1, 512], F32R)              # -mu*rstd row

            # DRAM scratch
            scr = dram.tile([MT * NS, 2, 512], F32)     # denom bounce
            cc_in = dram.tile([NS, 2, 512], F32R)
            cc_out = dram.tile([NS, GROUPS, 2, 512], F32R)

            p1sb_cm = tc.tile_pool(name="p1sb", bufs=1)
            p1sb = p1sb_cm.__enter__()
            hsT = p1sb.tile([128, KT, S], F32R)
            wq = p1sb.tile([128, KT, MT, 128], F32R)
            wk = p1sb.tile([128, KT, MT, 128], F32R)
            wv = p1sb.tile([128, KT, DC], F32R)
            wp = p1sb.tile([128, KT, MT, 128], F32R)

            # ---------------- input DMAs ----------------
            # Bulk tensors on the SP queue, ordered so the first score
            # matmul's deps land first. hsT is chunked along S.
            hsT_t = hsT_d.rearrange("(t p) s -> p t s", p=128)
            wq_t = wqT_d.rearrange("(t p) (m f) -> p t m f", p=128, f=128)
            wk_t = wkT_d.rearrange("(t p) (m f) -> p t m f", p=128, f=128)
            wv_t = wvT_d.rearrange("(t p) c -> p t c", p=128)
            wp_t = wpT_d.rearrange("(t p) (m f) -> p t m f", p=128, f=128)

            hsbp_cm = tc.tile_pool(name="hsbp", bufs=2)
            hsbp = hsbp_cm.__enter__()

            def wload(w_sb, src_ap, wb, eng="v"):
                nc.sync.dma_start(out=wb, in_=src_ap)
                # upcast bf16 -> fp32r; the startup chain is split across
                # DVE and the (still idle) ACT engine
                if eng == "a":
                    nc.scalar.activation(out=w_sb, in_=wb, func=AF.Copy)
                else:
                    nc.vector.tensor_scalar_mul(
                        out=w_sb, in0=wb, scalar1=1.0)

            CW = 128

            def hs_chunk(sn, eng="v"):
                sl2 = slice(sn * CW, (sn + 1) * CW)
                hsb = hsbp.tile([128, KT, CW], BF16, name="hsb")
                nc.sync.dma_start(out=hsb, in_=hsT_t[:, :, sl2])
                wload_cast(hsT[:, :, sl2], hsb, eng)

            def wload_cast(dst, srcb, eng):
                if eng == "a":
                    nc.scalar.activation(out=dst, in_=srcb, func=AF.Copy)
                else:
                    nc.vector.tensor_scalar_mul(
                        out=dst, in0=srcb, scalar1=1.0)

            wload(wk[:, :, 0, :], wk_t[:, :, 0, :],
                  hsbp.tile([128, KT, CW], BF16, name="hsb"), "v")
            wload(wq[:, :, 0, :], wq_t[:, :, 0, :],
                  hsbp.tile([128, KT, CW], BF16, name="hsb"), "a")
            for sn in range(4):
                hs_chunk(sn, "va"[sn % 2])
            wload(wv[:, :, 0:128], wv_t[:, :, 0:128],
                  hsbp.tile([128, KT, CW], BF16, name="hsb"), "v")
            wload(wv[:, :, 128:256], wv_t[:, :, 128:256],
                  hsbp.tile([128, KT, CW], BF16, name="hsb"), "a")

            def load_rest():
                for sn in range(4, 16):
                    hs_chunk(sn)
                wload(wp[:, :, 0, :], wp_t[:, :, 0, :],
                      hsbp.tile([128, KT, CW], BF16, name="hsb"))
                wload(wp[:, :, 1, :], wp_t[:, :, 1, :],
                      hsbp.tile([128, KT, CW], BF16, name="hsb"))
                wload(wk[:, :, 1, :], wk_t[:, :, 1, :],
                      hsbp.tile([128, KT, CW], BF16, name="hsb"))
                wload(wq[:, :, 1, :], wq_t[:, :, 1, :],
                      hsbp.tile([128, KT, CW], BF16, name="hsb"))

            # small constants on the gpsimd (SWDGE) queue
            nc.gpsimd.dma_start(
                out=bq_s, in_=bq_d.rearrange("(m p) -> p m", p=128))
            nc.gpsimd.dma_start(
                out=bk_s, in_=bk_d.rearrange("(m p) -> p m", p=128))
            nc.gpsimd.dma_start(out=gmr, in_=gbr_d[0:1, :, :])
            nc.gpsimd.dma_start(
                out=bt_s,
                in_=gbr_d[1:2, :, :].bitcast(F32).rearrange(
                    "r m p -> (r p) m"))
            nc.gpsimd.dma_start(out=bv_b, in_=bass.AP(
                tensor=bvh_d[:].tensor, offset=0, ap=[[0, 128], [1, DC]]))
            nc.vector.memset(eps_s, EPS)
            nc.vector.memset(cst[:, 7:8], 1.0)
            nc.vector.memset(cst[:, 8:9], 1.0 / D)
            # f32 -> f32r rounding casts via SWDGE dma (engine memset to an
            # f32r tile fails BIR verification)
            nc.gpsimd.dma_start(out=oD, in_=cst[0:4, 8:9])
            nc.gpsimd.dma_start(out=onesc, in_=cst[:, 7:8])
            nc.vector.memset(vA[:, :, :, HD:HD + 1].bitcast(F32), 1.0)
            onesr = onesc

            with (
                tc.tile_pool(name="pps", bufs=2, space="PSUM") as pps,
                tc.tile_pool(name="scps", bufs=2, space="PSUM") as scps,
                tc.tile_pool(name="ctxps", bufs=2, space="PSUM") as ctxps,
                tc.tile_pool(name="ptp", bufs=2) as ptp,
                tc.tile_pool(name="small", bufs=1) as small,
                tc.tile_pool(name="stg", bufs=1) as stg,
                tc.tile_pool(name="x2p", bufs=1) as x2p,
            ):
                def proj_group(w_sb, m, n, bias, out_sb):
                    """One [128,512] output block of a W-stationary proj."""
                    ps = pps.tile([128, 512], F32, name="gps")
                    for k in range(KT):
                        nc.tensor.matmul(
                            out=ps, lhsT=w_sb[:, k, m, :],
                            rhs=hsT[:, k, n * 512:(n + 1) * 512],
                            start=(k == 0), stop=(k == KT - 1))
                    o = out_sb[:, m, n * 512:(n + 1) * 512]
                    if bias is not None:
                        nc.vector.tensor_scalar_add(out=o, in0=ps, scalar1=bias)
                    else:
                        # x feeds an fp32r matmul: every write into x must
                        # carry an fp32r output dtype for BIR verification
                        nc.vector.tensor_scalar_add(
                            out=o.bitcast(F32R), in0=ps, scalar1=0.0)

                def v_group(j):
                    """V (natural layout) for s-tile j, hs stationary."""
                    ps = pps.tile([128, 512], F32, name="gps")
                    for k in range(KT):
                        nc.tensor.matmul(
                            out=ps[:, 0:DC],
                            lhsT=hsT[:, k, j * 128:(j + 1) * 128],
                            rhs=wv[:, k, :],
                            start=(k == 0), stop=(k == KT - 1))
                    nc.vector.tensor_add(
                        out=vA[:, j, :, 0:HD],
                        in0=ps[:, 0:DC].rearrange("p (h d) -> p h d", d=HD),
                        in1=bv_b.rearrange("p (h d) -> p h d", d=HD))

                def g_q(m, n):
                    return lambda: proj_group(wq, m, n, bq_s[:, m:m + 1], qT)

                def g_k(m, n):
                    return lambda: proj_group(wk, m, n, bk_s[:, m:m + 1], kT)

                def g_r(m, n):
                    return lambda: proj_group(wp, m, n, None, x)

                def g_v(j):
                    return lambda: v_group(j)

                # upfront: only what the first score matmul needs
                g_q(0, 0)()
                g_k(0, 0)()
                load_rest()

                # filler schedule per (qn, hp) block
                fillers = {
                    (0, 0): [g_v(0), g_v(1), g_v(2), g_v(3), g_k(0, 1),
                             g_v(4), g_v(5), g_v(6), g_v(7), g_k(0, 2),
                             g_v(8), g_v(9), g_v(10), g_v(11), g_k(0, 3),
                             g_v(12), g_v(13), g_v(14), g_v(15), g_r(0, 0),
                             g_q(1, 0), g_k(1, 0)],
                    (0, 1): [g_k(1, 1), g_k(1, 2), g_k(1, 3), g_r(1, 0),
                             g_q(0, 1), g_r(0, 1)],
                    (1, 0): [g_q(1, 1), g_r(1, 1)],
                    (1, 1): [g_q(0, 2), g_r(0, 2)],
                    (2, 0): [g_q(1, 2), g_r(1, 2)],
                    (2, 1): [g_q(0, 3), g_r(0, 3)],
                    (3, 0): [g_q(1, 3), g_r(1, 3)],
                    (3, 1): [],
                }

                def division(hp, qn, ctx0, ctx1):
                    """x[:, hp, qs] += ctx/denom (x holds res).

                    The ctx psum banks are staged to SBUF so the next
                    block's ctx matmuls aren't blocked on the denominator
                    bounce — except for the very last block, where no
                    successor exists and the shorter chain wins."""
                    qs = slice(qn * 512, (qn + 1) * 512)
                    blk = qn * MT + hp
                    last = (qn == NS - 1 and hp == MT - 1)
                    if last:
                        c0, c0sl = ctx0, (slice(0, HD), slice(0, 512))
                        c1 = ctx1
                        d0, d1 = ctx0[HD:HD + 1, :], ctx1[HD:HD + 1, :]
                    else:
                        ctxc = stg.tile([128, 1024], F32, name="ctxc")
                        nc.vector.tensor_copy(
                            out=ctxc[0:HD + 1, 0:512], in_=ctx0[0:HD + 1, :])
                        nc.vector.tensor_copy(
                            out=ctxc[0:HD + 1, 512:1024],
                            in_=ctx1[0:HD + 1, :])
                        d0 = ctxc[HD:HD + 1, 0:512]
                        d1 = ctxc[HD:HD + 1, 512:1024]
                    rr = small.tile([1, 1024], F32, name="rr")
                    nc.vector.reciprocal(out=rr[:, 0:512], in_=d0)
                    nc.vector.reciprocal(out=rr[:, 512:1024], in_=d1)
                    nc.sync.dma_start(
                        out=bass.AP(tensor=scr.tensor,
                                    offset=scr.offset + blk * 1024,
                                    ap=[[1, 1024]]),
                        in_=rr)
                    rbs = stg.tile([64, 1024], F32, name="rbs")
                    nc.sync.dma_start(
                        out=rbs,
                        in_=bass.AP(tensor=scr.tensor,
                                    offset=scr.offset + blk * 1024,
                                    ap=[[0, 64], [512, 2], [1, 512]]))
                    tmp = stg.tile([128, 512], F32, name="tmp")
                    if last:
                        nc.vector.tensor_mul(
                            out=tmp[0:64, :], in0=ctx0[0:HD, :],
                            in1=rbs[:, 0:512])
                        nc.vector.tensor_mul(
                            out=tmp[64:128, :], in0=ctx1[0:HD, :],
                            in1=rbs[:, 512:1024])
                    else:
                        nc.vector.tensor_mul(
                            out=tmp[0:64, :], in0=ctxc[0:HD, 0:512],
                            in1=rbs[:, 0:512])
                        nc.vector.tensor_mul(
                            out=tmp[64:128, :], in0=ctxc[0:HD, 512:1024],
                            in1=rbs[:, 512:1024])
                    nc.vector.tensor_add(
                        out=x[:, hp, qs].bitcast(F32R), in0=x[:, hp, qs],
                        in1=tmp)

                def attention_block(hp, qn):
                    qs = slice(qn * 512, (qn + 1) * 512)
                    ctx0 = ctxps.tile([128, 512], F32, name="ctx")
                    ctx1 = ctxps.tile([128, 512], F32, name="ctx")
                    fl = list(fillers[(qn, hp)])
                    # spread fillers evenly over the 16 ks slots
                    per_slot = [0] * ST
                    for i in range(len(fl)):
                        per_slot[(i * ST) // max(1, len(fl))] += 1
                    fl.reverse()

                    def ctx_mms(pt, ks):
                        nc.tensor.matmul(
                            out=ctx0[0:HD + 1, :],
                            lhsT=vA[:, ks, 2 * hp, :],
                            rhs=pt[:, 0:512],
                            start=(ks == 0), stop=(ks == ST - 1))
                        nc.tensor.matmul(
                            out=ctx1[0:HD + 1, :],
                            lhsT=vA[:, ks, 2 * hp + 1, :],
                            rhs=pt[:, 512:1024],
                            start=(ks == 0), stop=(ks == ST - 1))

                    prev = None
                    for ks in range(ST):
                        sc = scps.tile([128, 1024], F32, name="sc")
                        kslc = slice(ks * 128, (ks + 1) * 128)
                        nc.tensor.matmul(
                            out=sc[:, 0:512],
                            lhsT=kT[0:64, hp, kslc],
                            rhs=qT[0:64, hp, qs])
                        nc.tensor.matmul(
                            out=sc[:, 512:1024],
                            lhsT=kT[64:128, hp, kslc],
                            rhs=qT[64:128, hp, qs])
                        pt = ptp.tile([128, 1024], F32R, name="pt")
                        nc.scalar.activation(
                            out=pt, in_=sc, func=AF.Exp,
                            scale=float(1.0 / np.sqrt(HD)))
                        for _ in range(per_slot[ks]):
                            if fl:
                                fl.pop()()
                        if prev is not None:
                            ctx_mms(*prev)
                        prev = (pt, ks)
                    ctx_mms(*prev)
                    division(hp, qn, ctx0, ctx1)

                stats_ctx = {}

                def stats_part(qn, t):
                    """relu + square + stats-matmul contribution of head
                    pair t, emitted right after division(t, qn). The psum
                    rows are merged into the SBUF accumulator immediately
                    so the psum pool is never held across a block."""
                    qs = slice(qn * 512, (qn + 1) * 512)
                    if t == 0:
                        stats_ctx[qn] = (
                            small.tile([1, 1024], F32R, name="st"),)
                    (st,) = stats_ctx[qn]
                    x2 = x2p.tile([128, 512], F32R, name="x2")
                    nc.vector.tensor_scalar_max(
                        out=x[:, t, qs].bitcast(F32R), in0=x[:, t, qs],
                        scalar1=0.0)
                    nc.scalar.activation(
                        out=x2, in_=x[:, t, qs], func=AF.Square)
                    sp = pps.tile([128, 512], F32, name="gps")
                    sq = pps.tile([128, 512], F32, name="gps")
                    nc.tensor.matmul(
                        out=sp[0:1, :], lhsT=onesr,
                        rhs=x[:, t, qs].bitcast(F32R))
                    nc.tensor.matmul(
                        out=sq[0:1, :], lhsT=onesr, rhs=x2)
                    if t == 0:
                        nc.vector.tensor_scalar_mul(
                            out=st[:, 0:512], in0=sp[0:1, :], scalar1=1.0)
                        nc.vector.tensor_scalar_mul(
                            out=st[:, 512:1024], in0=sq[0:1, :],
                            scalar1=1.0)
                    else:
                        nc.vector.tensor_add(
                            out=st[:, 0:512],
                            in0=st[:, 0:512].bitcast(F32),
                            in1=sp[0:1, :])
                        nc.vector.tensor_add(
                            out=st[:, 512:1024],
                            in0=st[:, 512:1024].bitcast(F32),
                            in1=sq[0:1, :])

                def stats(qn):
                    """bounce accumulated stats to DRAM + AllGather."""
                    (st,) = stats_ctx.pop(qn)
                    nc.sync.dma_start(
                        out=bass.AP(tensor=cc_in.tensor,
                                    offset=cc_in.offset + qn * 1024,
                                    ap=[[1, 1024]]),
                        in_=st)
                    nc.gpsimd.collective_compute(
                        "AllGather", ALU.bypass,
                        replica_groups=REPLICA_GROUPS,
                        ins=[cc_in[qn].opt()], outs=[cc_out[qn].opt()],
                    )

                def apply_ln(qn):
                    """Reduce gathered stats (matmul), row math on [1,512],
                    broadcast gamma*A / gamma*B+beta rows (matmuls), apply,
                    DMA out."""
                    qs = slice(qn * 512, (qn + 1) * 512)
                    cc_sb = stg.tile([4, 1024], F32R, name="ccsb")
                    nc.sync.dma_start(out=cc_sb, in_=cc_out[qn].rearrange(
                        "c v s -> c (v s)"))
                    stt = pps.tile([128, 512], F32, name="gps")
                    stq = pps.tile([128, 512], F32, name="gps")
                    nc.tensor.matmul(
                        out=stt[0:1, :], lhsT=oD,
                        rhs=cc_sb[:, 0:512])
                    nc.tensor.matmul(
                        out=stq[0:1, :], lhsT=oD,
                        rhs=cc_sb[:, 512:1024])
                    # row math on [1, 512]: stt[0]=mu, stt[32]=E[x^2];
                    # rm is reused in place down the chain
                    rm = small.tile([1, 512], F32, name="rm")
                    nc.scalar.activation(
                        out=rm, in_=stt[0:1, :], func=AF.Square)
                    nc.vector.scalar_tensor_tensor(
                        out=rm, in0=stq[0:1, :], scalar=1.0,
                        in1=rm, op0=ALU.mult, op1=ALU.subtract)
                    nc.scalar.activation(
                        out=rm, in_=rm, func=AF.Sqrt, bias=eps_s[0:1, :])
                    arow = small.tile([1, 512], F32R, name="arow")
                    nc.vector.reciprocal(out=arow, in_=rm)
                    # brow = -mu * rstd
                    nc.vector.scalar_tensor_tensor(
                        out=brow, in0=stt[0:1, :], scalar=-1.0,
                        in1=arow.bitcast(F32), op0=ALU.mult, op1=ALU.mult)
                    # broadcast rows with gamma/beta folded in:
                    #   ab[:,0:512] = gamma[p]*A[s]; ab[:,512:]=gamma*B+beta
                    ot = stg.tile([128, MT, 512], BF16, name="ot")
                    for t in range(MT):
                        abA = pps.tile([128, 512], F32, name="gps")
                        abB = pps.tile([128, 512], F32, name="gps")
                        nc.tensor.matmul(
                            out=abA, lhsT=gmr[:, t, :], rhs=arow)
                        nc.tensor.matmul(
                            out=abB, lhsT=gmr[:, t, :], rhs=brow)
                        ota = stg.tile([128, 512], F32, name="tmp")
                        nc.vector.tensor_mul(
                            out=ota, in0=x[:, t, qs], in1=abA)
                        # ot = (ota + beta) + gamma*brow_bcast
                        nc.vector.scalar_tensor_tensor(
                            out=ot[:, t, :], in0=ota,
                            scalar=bt_s[:, t:t + 1], in1=abB,
                            op0=ALU.add, op1=ALU.add)
                        out_t = out_d.rearrange("(t p) s -> p t s", p=128)
                        nc.sync.dma_start(
                            out=out_t[:, t, qs], in_=ot[:, t, :])

                # ================= main loop =================
                for qn in range(NS):
                    for hp in range(MT):
                        attention_block(hp, qn)
                        stats_part(qn, hp)
                    stats(qn)
                    if qn == 1:
                        apply_ln(0)
                # applies 1 and 2 run inside AllGather(3)'s flight window
                apply_ln(1)
                apply_ln(2)
                apply_ln(NS - 1)
            hsbp_cm.__exit__(None, None, None)
            p1sb_cm.__exit__(None, None, None)
    lp_cm.__exit__(None, None, None)
    _split_waits(nc)
    return nc


_NC = None
LAST_RESULT = None


def _get_nc():
    global _NC
    if _NC is None:
        _NC = build_bass()
    return _NC


def kernel(hidden_states, Wq, bq, Wk, bk, Wv, bv, Wp, gamma, beta):
    hs = np.ascontiguousarray(np.asarray(hidden_states, dtype=np.float32))
    Wq = np.asarray(Wq, np.float32)
    Wk = np.asarray(Wk, np.float32)
    Wv = np.asarray(Wv, np.float32)
    Wp = np.asarray(Wp, np.float32)
    bq = np.asarray(bq, np.float32)
    bk = np.asarray(bk, np.float32)
    bv = np.asarray(bv, np.float32)
    gamma = np.asarray(gamma, np.float32)
    beta = np.asarray(beta, np.float32)

    nc = _get_nc()
    in_maps = []
    for c in range(NCORES):
        b, g = divmod(c, GROUPS)
        sl = slice(g * DC, (g + 1) * DC)
        gb = np.stack([gamma[sl].reshape(MT, 128),
                       beta[sl].reshape(MT, 128)])  # [2, MT, 128]
        in_maps.append({
            "hsT": np.ascontiguousarray(hs[b].T).astype(
                ml_dtypes.bfloat16),
            "wqT": np.ascontiguousarray(Wq[sl].T).astype(ml_dtypes.bfloat16),
            "wkT": np.ascontiguousarray(Wk[sl].T).astype(ml_dtypes.bfloat16),
            "wvT": np.ascontiguousarray(Wv[sl].T).astype(ml_dtypes.bfloat16),
            "wpT": np.ascontiguousarray(Wp[sl].T).astype(ml_dtypes.bfloat16),
            "bq": np.ascontiguousarray(bq[sl]),
            "bk": np.ascontiguousarray(bk[sl]),
            "bv": np.ascontiguousarray(bv[sl]).astype(ml_dtypes.bfloat16),
            "gbrows": np.ascontiguousarray(gb),
        })
    res = run_bass_kernel_spmd(nc, in_maps, core_ids=list(range(NCORES)))
    global LAST_RESULT
    LAST_RESULT = res
    out = np.empty((B, S, D), np.float32)
    for c, r in enumerate(res.results):
        b, g = divmod(c, GROUPS)
        out[b, :, g * DC:(g + 1) * DC] = np.asarray(
            r["outT"]).astype(np.float32).T
    return out
